# revision 29
# baseline (speedup 1.0000x reference)
"""Trainium2 Bass kernel for nn_BinaryClassifier (CNN + 2-qubit circuit head).

Data-parallel over 8 cores (65536 -> 8192/core), NT=1024 images per tile.

Structure per tile:
  conv1: 12 blocks (4y x 12x out, patch 8x16 = K 128 full), 1 matmul each.
         Bias folded into the PSUM drain (tensor_scalar w/ AP bias).
  pool1: drain -> stage1 max(yp halves, cross-partition-offset TT)
         -> stage2 max+relu (STT) writing 32-row chunks into conv2 K-tiles.
  conv2: 8 out-tiles x 2 accumulated matmuls over [128,N] K-tiles
         (zero weight rows where a row is outside the 6x8 patch).
  pool2: same pattern -> fc1 rhs tiles F2a/F2b.
  fc1:   2 matmuls + Act relu/bias drain.  fc2: 8 small matmuls.
  head:  quantum circuit reduced to 6 cosines, computed once after the loop.
"""
import os, sys
sys.path.insert(0, "/opt/trn_rl_repo")
import numpy as np
import ml_dtypes

from concourse import bass, tile, bacc
from concourse import mybir
from concourse.bass_utils import run_bass_kernel_spmd

dt = mybir.dt
AF = mybir.ActivationFunctionType
ALU = mybir.AluOpType

B = int(os.environ.get("BASS_KERNEL_B", "65536"))
NCORES = int(os.environ.get("BASS_KERNEL_CORES", "8"))
BC = B // NCORES          # images per core
NT = min(1024, BC)        # images per tile
NTILES = BC // NT
NSUB = NT // 128          # fc2 image-subtiles per tile

# conv1 blocking: 12 blocks = 6 by (4 out rows each) x 2 bx (12 out cols each)
# patch per block: 8 rows x 16 cols = 128 K-rows (full)
# M layout: quad (yp, xq) * 32 + payload; payload = pc*4 + pr*2 + ch (24 used)


def _bf16(a):
    return np.asarray(a, dtype=np.float32).astype(np.float16)


# ---------------------------------------------------------------- host packing

def build_a(x):
    """x: [B, 784] -> [12, 128, B] bf16 im2col (block, patch-pixel, image)."""
    n = x.shape[0]
    xb = _bf16(x).reshape(n, 28, 28)
    out = np.empty((12, 128, n), dtype=np.float16)
    for by in range(6):
        for bx in range(2):
            b = by * 2 + bx
            for iy in range(8):
                src = xb[:, 4 * by + iy, 12 * bx:12 * bx + 16]   # [n,16]
                out[b, iy * 16:iy * 16 + 16, :] = src.T
    return out


def _m_decode(m):
    """conv1 M index -> (valid, dy, dx, ch): out pixel local coords."""
    quad, pay = m >> 5, m & 31
    if pay >= 24:
        return False, 0, 0, 0
    yp, xq = quad >> 1, quad & 1
    pc, r = pay >> 2, pay & 3
    pr, ch = r >> 1, r & 1
    return True, 2 * pr + yp, 2 * pc + xq, ch


def build_w1(w1):
    """w1: [2,1,5,5] -> [128, 12*128] bf16 (same lhsT for every block)."""
    W = np.zeros((128, 128), dtype=np.float32)
    w1 = np.asarray(w1, dtype=np.float32).reshape(2, 5, 5)
    for m in range(128):
        ok, dy, dx, ch = _m_decode(m)
        if not ok:
            continue
        for ky in range(5):
            for kx in range(5):
                W[(dy + ky) * 16 + (dx + kx), m] = w1[ch, ky, kx]
    return _bf16(W)


def build_w2(w2):
    """w2: [16,2,5,5] -> [128, 16*128] bf16.

    mm i = j*2 + bxi, j = (a, xh) = 2*a + xh.  K-tile (yh(j), bxi):
    K-row k = 32*slot + pc*4 + pr*2 + ch  (slot = chunk index 0..3)
      chunk by = slot + 2*yh;  pooled R = 2*by + pr, C = 6*bxi + pc.
    M col m = quad(yp2, xp2)*32 + oc*2 + bb:
      out pixel oy = 2*a + yp2, ox = 4*xh + 2*bb + xp2.
    """
    W = np.zeros((128, 16 * 128), dtype=np.float32)
    w2 = np.asarray(w2, dtype=np.float32)
    for a in range(4):
        yh = 0 if a < 2 else 1
        for xh in range(2):
            j = 2 * a + xh
            for bxi in range(2):
                col0 = (j * 2 + bxi) * 128
                for slot in range(4):
                    by = slot + 2 * yh
                    for pc in range(6):
                        for pr in range(2):
                            for ch in range(2):
                                k = 32 * slot + pc * 4 + pr * 2 + ch
                                R = 2 * by + pr
                                C = 6 * bxi + pc
                                for m in range(128):
                                    quad, pay = m >> 5, m & 31
                                    yp2, xp2 = quad >> 1, quad & 1
                                    oc, bb = pay >> 1, pay & 1
                                    oy = 2 * a + yp2
                                    ox = 4 * xh + 2 * bb + xp2
                                    ky, kx = R - oy, C - ox
                                    if 0 <= ky < 5 and 0 <= kx < 5:
                                        W[k, col0 + m] = w2[oc, ch, ky, kx]
    return _bf16(W)


def build_wfc1(fc1_w):
    """fc1_w: [64, 256] -> [128, 2*64] bf16 (2 K-tiles F2a/F2b)."""
    W = np.zeros((128, 2 * 64), dtype=np.float32)
    fc1_w = np.asarray(fc1_w, dtype=np.float32)
    for t in range(2):
        for p in range(128):
            jl, pay = p >> 5, p & 31
            j = 4 * t + jl
            a, xh = j >> 1, j & 1
            oc, bb = pay >> 1, pay & 1
            b_ = 2 * xh + bb
            flat = oc * 16 + a * 4 + b_
            W[p, t * 64:t * 64 + 64] = fc1_w[:, flat]
    return _bf16(W)


def head_constants(qnn_params, fc3_w, fc3_b, fc2_b):
    """Reduce the 2-qubit circuit tail + fc3 to z = c0 + sum Mk*cos(...)."""
    p = np.asarray(qnn_params, dtype=np.float64)

    def ry(t):
        c, s = np.cos(t), np.sin(t)
        return np.array([[c, -s], [s, c]])

    def kron_w0(U):
        return np.kron(U, np.eye(2))

    def kron_w1(U):
        return np.kron(np.eye(2), U)

    CN01 = np.zeros((4, 4)); CN01[0, 0] = CN01[1, 1] = 1; CN01[2, 3] = CN01[3, 2] = 1
    CN10 = np.zeros((4, 4))
    for q0 in range(2):
        for q1 in range(2):
            CN10[((q0 ^ q1) * 2 + q1), q0 * 2 + q1] = 1
    U = np.eye(4)
    U = kron_w0(ry(p[0])) @ U
    U = kron_w1(ry(p[1])) @ U
    U = CN01 @ U
    U = kron_w0(ry(p[2])) @ U
    U = kron_w1(ry(p[3])) @ U
    U = CN10 @ U
    U = kron_w0(ry(p[4])) @ U
    U = kron_w1(ry(p[5])) @ U
    U = CN01 @ U
    U = kron_w0(ry(p[6])) @ U
    U = kron_w1(ry(p[7])) @ U
    S = np.diag([1.0, -1.0, -1.0, 1.0])
    M = 0.25 * (U.T @ S @ U)
    w3 = float(np.asarray(fc3_w).reshape(()))
    b3 = float(np.asarray(fc3_b).reshape(()))
    c0 = float(np.trace(M)) * w3 + b3
    k = {
        "A": 2 * M[0, 3] * w3,   # cos(2x0+2x1)
        "B": 2 * M[1, 2] * w3,   # cos(2x0-2x1)
        "C": 2 * M[0, 2] * w3,   # cos(2x0+2ang)
        "D": 2 * M[1, 3] * w3,   # cos(2x0-2ang)
        "E": 2 * M[0, 1] * w3,   # cos(2x1+2ang)
        "F": 2 * M[2, 3] * w3,   # cos(2x1-2ang)
    }
    return c0, k, float(fc2_b[0]), float(fc2_b[1])


# ---------------------------------------------------------------- bass program

def build_program(weights):
    nc = bacc.Bacc(None, target_bir_lowering=False, debug=False)
    a_d = nc.declare_dram_parameter("a_c1", [12, 128, BC], dt.float16, isOutput=False)
    w1_d = nc.declare_dram_parameter("w1", [128, 128], dt.float16, isOutput=False)
    w2_d = nc.declare_dram_parameter("w2", [128, 16 * 128], dt.float16, isOutput=False)
    wf1_d = nc.declare_dram_parameter("wf1", [128, 2 * 64], dt.float16, isOutput=False)
    wf2_d = nc.declare_dram_parameter("wf2", [64, 2], dt.float16, isOutput=False)
    cst_d = nc.declare_dram_parameter("cst", [128, 16], dt.float32, isOutput=False)
    y_d = nc.declare_dram_parameter("y", [2, BC], dt.float32, isOutput=True)

    c0, K, b20, b21 = weights["head"]
    pi = float(np.pi)

    with tile.TileContext(nc) as tc:
        with tc.tile_pool(name="cw", bufs=1) as cw, \
             tc.tile_pool(name="sx", bufs=2) as sx, \
             tc.tile_pool(name="sc", bufs=3) as sc, \
             tc.tile_pool(name="sm", bufs=2) as sm, \
             tc.tile_pool(name="se", bufs=2) as se, \
             tc.tile_pool(name="sf", bufs=2) as sf, \
             tc.tile_pool(name="hd", bufs=1) as hd, \
             tc.tile_pool(name="p1", bufs=4, space="PSUM") as p1:

            W1 = cw.tile([128, 128], dt.float16)
            nc.sync.dma_start(out=W1[:], in_=w1_d[:])
            W2 = cw.tile([128, 16 * 128], dt.float16)
            nc.sync.dma_start(out=W2[:], in_=w2_d[:])
            WF1 = cw.tile([128, 2 * 64], dt.float16)
            nc.sync.dma_start(out=WF1[:], in_=wf1_d[:])
            WF2 = cw.tile([64, 2], dt.float16)
            nc.sync.dma_start(out=WF2[:], in_=wf2_d[:])
            CST = cw.tile([128, 16], dt.float32)
            nc.sync.dma_start(out=CST[:], in_=cst_d[:])

            # engine rotation: drains on DVE/Act (GpSimd cannot read PSUM),
            # SBUF-only pool stages on DVE/GpSimd
            drain_eng = [nc.scalar]
            tt_eng = [nc.vector, nc.vector]

            for it in range(NTILES):
                n0 = it * NT
                xc = sx.tile([128, 12 * NT], dt.float16)
                nc.sync.dma_start(
                    out=xc[:].rearrange("p (b n) -> p b n", b=12),
                    in_=a_d[:, :, n0:n0 + NT].transpose([1, 0, 2]))

                # T K-tiles for conv2: [yh] each [128, 2*NT], free = (bx, n)
                T = [se.tile([128, 2 * NT], dt.float16, tag=f"T{yh}",
                             name=f"T{yh}") for yh in range(2)]

                di = 0
                ti = 0
                # conv1: block pairs (by, bx=0/1) share weights -> one matmul
                # of N=2*NT into a 4-bank psum tile
                for by in range(6):
                    psu = [p1.tile([128, NT], dt.float32, tag="ps", name=f"ps{by}{h}")
                           for h in range(2)]
                    for q in range(2 * NT // 512):
                        c0_ = 2 * by * NT + q * 512
                        ph_, qh = q // (NT // 512), q % (NT // 512)
                        nc.tensor.matmul(out=psu[ph_][:, qh * 512:(qh + 1) * 512],
                                         lhsT=W1[:, 0:128],
                                         rhs=xc[:, c0_:c0_ + 512],
                                         start=True, stop=True)
                    # drain + conv1 bias + relu (per-partition AP scalar)
                    C = sc.tile([128, 2 * NT], dt.float16, tag="C")
                    for h in range(2):
                        eng = drain_eng[di % len(drain_eng)]; di += 1
                        if eng is nc.scalar:
                            nc.scalar.activation(out=C[:, h * NT:(h + 1) * NT],
                                                 in_=psu[h][:], func=AF.Relu,
                                                 bias=CST[:, 0:1])
                        else:
                            eng.tensor_scalar(out=C[:, h * NT:(h + 1) * NT],
                                              in0=psu[h][:],
                                              scalar1=CST[:, 0:1], scalar2=0.0,
                                              op0=ALU.add, op1=ALU.max)
                    # stage1: max over yp; upper half copied to base-0 first
                    # (two-input SBUF ops require equal base partitions)
                    S1 = sm.tile([64, 2 * NT], dt.float16, tag="S1")
                    nc.scalar.dma_start(out=S1[:], in_=C[64:128, :])
                    M1 = sm.tile([64, 2 * NT], dt.float16, tag="M1")
                    eng = tt_eng[ti % 2]; ti += 1
                    eng.tensor_tensor(out=M1[:], in0=C[0:64, :], in1=S1[:],
                                      op=ALU.max)
                    S2 = sm.tile([32, 2 * NT], dt.float16, tag="S2")
                    nc.scalar.dma_start(out=S2[:], in_=M1[32:64, :])
                    # stage2: max over xq + relu -> chunk row of T (both blocks)
                    yh = 0 if by < 4 else 1
                    slot = by - 2 * yh
                    eng = tt_eng[ti % 2]; ti += 1
                    eng.tensor_tensor(
                        out=T[yh][32 * slot:32 * slot + 32, :],
                        in0=M1[0:32, :], in1=S2[:], op=ALU.max)

                # chunks by=2,3 (T0 slots 2,3) also open T1 as slots 0,1
                nc.scalar.dma_start(out=T[1][0:64, :], in_=T[0][64:128, :])

                # conv2: 8 out-tiles x 2 accumulated matmuls; pairs (j, j+1)
                # share one 4-bank psum tile and batched drain/stage1
                F2 = [sf.tile([128, NT], dt.float16, tag=f"F2{t}", name=f"F2{t}")
                      for t in range(2)]
                for a_ in range(4):
                    yh = 0 if a_ < 2 else 1
                    ps2 = [p1.tile([128, NT], dt.float32, tag="ps", name=f"p2{a_}{h}")
                           for h in range(2)]
                    for xh in range(2):
                        j = 2 * a_ + xh
                        for bxi in range(2):
                            for q in range(NT // 512):
                                nc.tensor.matmul(
                                    out=ps2[xh][:, q * 512:(q + 1) * 512],
                                    lhsT=W2[:, (j * 2 + bxi) * 128:(j * 2 + bxi + 1) * 128],
                                    rhs=T[yh][:, bxi * NT + q * 512:bxi * NT + (q + 1) * 512],
                                    start=(bxi == 0), stop=(bxi == 1))
                    D = sc.tile([128, 2 * NT], dt.float16, tag="D")
                    for h in range(2):
                        eng = drain_eng[di % len(drain_eng)]; di += 1
                        if eng is nc.scalar:
                            nc.scalar.activation(out=D[:, h * NT:(h + 1) * NT],
                                                 in_=ps2[h][:], func=AF.Relu,
                                                 bias=CST[:, 1:2])
                        else:
                            eng.tensor_scalar(out=D[:, h * NT:(h + 1) * NT],
                                              in0=ps2[h][:],
                                              scalar1=CST[:, 1:2], scalar2=0.0,
                                              op0=ALU.add, op1=ALU.max)
                    S1b = sm.tile([64, 2 * NT], dt.float16, tag="S1b")
                    nc.scalar.dma_start(out=S1b[:], in_=D[64:128, :])
                    M2 = sm.tile([64, 2 * NT], dt.float16, tag="M2")
                    eng = tt_eng[ti % 2]; ti += 1
                    eng.tensor_tensor(out=M2[:], in0=D[0:64, :], in1=S1b[:],
                                      op=ALU.max)
                    S2b = sm.tile([32, 2 * NT], dt.float16, tag="S2b")
                    nc.scalar.dma_start(out=S2b[:], in_=M2[32:64, :])
                    for xh in range(2):
                        j = 2 * a_ + xh
                        ft, jl = j // 4, j % 4
                        eng = tt_eng[ti % 2]; ti += 1
                        eng.tensor_tensor(
                            out=F2[ft][32 * jl:32 * jl + 32, :],
                            in0=M2[0:32, xh * NT:(xh + 1) * NT],
                            in1=S2b[:, xh * NT:(xh + 1) * NT], op=ALU.max)

                # fc1 (K=256 over 2 tiles) -> relu -> F1 bf16
                psf = p1.tile([64, NT], dt.float32, tag="ps")
                for t_ in range(2):
                    for q in range(NT // 512):
                        nc.tensor.matmul(out=psf[:, q * 512:(q + 1) * 512],
                                         lhsT=WF1[:, 64 * t_:64 * (t_ + 1)],
                                         rhs=F2[t_][:, q * 512:(q + 1) * 512],
                                         start=(t_ == 0), stop=(t_ == 1))
                F1 = sf.tile([64, NT], dt.float16, tag="F1")
                nc.scalar.activation(out=F1[:], in_=psf[:, 0:NT], func=AF.Relu,
                                     bias=CST[0:64, 2:3])

                # fc2 img-major: NSUB matmuls N=2 -> psum [128, 2*NSUB]
                psg = p1.tile([128, NT], dt.float32, tag="ps")
                for s in range(NSUB):
                    nc.tensor.matmul(out=psg[:, 2 * s:2 * s + 2],
                                     lhsT=F1[:, 128 * s:128 * (s + 1)],
                                     rhs=WF2[:], start=True, stop=True)
                H = hd.tile([128, 2 * NSUB], dt.float32, tag="H", bufs=2)
                nc.vector.tensor_copy(out=H[:], in_=psg[:, 0:2 * NSUB])

                # ---- head (per tile): H [128, (s, c)] -> y [128, NSUB]
                NC_ = NSUB
                Hv = H[:].rearrange("p (u c) -> p u c", c=2)
                x0 = Hv[:, :, 0]
                x1 = Hv[:, :, 1]
                t0 = hd.tile([128, NC_], dt.float32, tag="t0", bufs=2)
                nc.vector.tensor_scalar(out=t0[:], in0=x0, scalar1=-1.0,
                                        scalar2=pi - b20, op0=ALU.mult, op1=ALU.add)
                t1 = hd.tile([128, NC_], dt.float32, tag="t1", bufs=2)
                nc.vector.tensor_scalar(out=t1[:], in0=x1, scalar1=-1.0,
                                        scalar2=pi - b21, op0=ALU.mult, op1=ALU.add)
                ang = hd.tile([128, NC_], dt.float32, tag="ang", bufs=2)
                nc.vector.tensor_tensor(out=ang[:], in0=t0[:], in1=t1[:], op=ALU.mult)

                qpi = pi / 4
                hb = {"A": b20 + b21 + qpi, "B": b20 - b21 + qpi,
                      "C": b20 + qpi, "D": b20 + qpi,
                      "E": b21 + qpi, "F": b21 + qpi}
                AR = hd.tile([128, 6 * NC_], dt.float32, tag="AR", bufs=2)
                plan = (("A", x0, x1, ALU.add), ("B", x0, x1, ALU.subtract),
                        ("C", x0, ang[:], ALU.add), ("D", x0, ang[:], ALU.subtract),
                        ("E", x1, ang[:], ALU.add), ("F", x1, ang[:], ALU.subtract))
                for i, (nm, a0, a1, op) in enumerate(plan):
                    nc.vector.scalar_tensor_tensor(
                        out=AR[:, NC_ * i:NC_ * (i + 1)], in0=a0, scalar=hb[nm],
                        in1=a1, op0=ALU.add, op1=op)
                tq = hd.tile([128, 6 * NC_], dt.float32, tag="tq", bufs=2)
                nc.vector.tensor_scalar(out=tq[:], in0=AR[:], scalar1=float(1 / pi),
                                        scalar2=None, op0=ALU.mult)
                ti_ = hd.tile([128, 6 * NC_], dt.int32, tag="ti", bufs=2)
                nc.vector.tensor_copy(out=ti_[:], in_=tq[:])
                tf_ = hd.tile([128, 6 * NC_], dt.float32, tag="tf", bufs=2)
                nc.vector.tensor_copy(out=tf_[:], in_=ti_[:])
                hh = hd.tile([128, 6 * NC_], dt.float32, tag="hh", bufs=2)
                nc.vector.scalar_tensor_tensor(out=hh[:], in0=tf_[:], scalar=-pi,
                                               in1=AR[:], op0=ALU.mult, op1=ALU.add)
                SN = hd.tile([128, 6 * NC_], dt.float32, tag="SN", bufs=2)
                nc.scalar.activation(out=SN[:], in_=hh[:], func=AF.Sin, scale=2.0)
                cosv = {nm: SN[:, NC_ * i:NC_ * (i + 1)]
                        for i, nm in enumerate("ABCDEF")}

                acc = hd.tile([128, NC_], dt.float32, tag="acc0", bufs=2)
                nc.vector.tensor_scalar(out=acc[:], in0=cosv["A"], scalar1=K["A"],
                                        scalar2=c0, op0=ALU.mult, op1=ALU.add)
                for i, nm in enumerate("BCDE"):
                    acc2 = hd.tile([128, NC_], dt.float32, tag=f"acc{i+1}", bufs=2)
                    nc.vector.scalar_tensor_tensor(out=acc2[:], in0=cosv[nm][:],
                                                   scalar=K[nm], in1=acc[:],
                                                   op0=ALU.mult, op1=ALU.add)
                    acc = acc2
                Yt = hd.tile([128, NC_], dt.float32, tag="Yt", bufs=2)
                nc.vector.scalar_tensor_tensor(out=Yt[:], in0=cosv["F"],
                                               scalar=K["F"], in1=acc[:],
                                               op0=ALU.mult, op1=ALU.add)

                # final: out0=-ln(1+e^{1-2y}), out1=-ln(1+e^{2y-1})
                V = hd.tile([128, NC_], dt.float32, tag="V", bufs=2)
                nc.scalar.activation(out=V[:], in_=Yt[:], func=AF.Exp,
                                     bias=CST[:, 8:9], scale=-2.0)
                Wr = hd.tile([128, NC_], dt.float32, tag="Wr", bufs=2)
                nc.vector.reciprocal(out=Wr[:], in_=V[:])
                L0 = hd.tile([128, NC_], dt.float32, tag="L0", bufs=2)
                nc.scalar.activation(out=L0[:], in_=V[:], func=AF.Ln,
                                     bias=CST[:, 8:9], scale=1.0)
                L1 = hd.tile([128, NC_], dt.float32, tag="L1", bufs=2)
                nc.scalar.activation(out=L1[:], in_=Wr[:], func=AF.Ln,
                                     bias=CST[:, 8:9], scale=1.0)
                O = hd.tile([128, 2 * NC_], dt.float32, tag="O", bufs=2)
                Ov = O[:].rearrange("p (c u) -> p c u", c=2)
                nc.vector.tensor_scalar(out=Ov[:, 0, :], in0=L0[:], scalar1=-1.0,
                                        scalar2=None, op0=ALU.mult)
                nc.vector.tensor_scalar(out=Ov[:, 1, :], in0=L1[:], scalar1=-1.0,
                                        scalar2=None, op0=ALU.mult)
                for c in range(2):
                    nc.sync.dma_start(
                        out=y_d[c, n0:n0 + NT].rearrange("(s p) -> p s", p=128),
                        in_=Ov[:, c, :].rearrange("p s -> p s"))

    nc.compile()
    return nc


def kernel(x, conv1_w, conv1_b, conv2_w, conv2_b, fc1_w, fc1_b,
           fc2_w, fc2_b, fc3_w, fc3_b, qnn_params):
    x = np.asarray(x, dtype=np.float32).reshape(B, 784)
    a = build_a(x)
    W1 = build_w1(conv1_w)
    W2 = build_w2(conv2_w)
    WF1 = build_wfc1(fc1_w)
    WF2 = _bf16(np.asarray(fc2_w, np.float32).T)  # [64, 2]
    c0, K, b20, b21 = head_constants(qnn_params, fc3_w, fc3_b,
                                     np.asarray(fc2_b, np.float32))
    cst = np.zeros((128, 16), dtype=np.float32)
    b1 = np.asarray(conv1_b, np.float32)
    b2 = np.asarray(conv2_b, np.float32)
    for p in range(128):
        pay = p & 31
        cst[p, 0] = b1[pay & 1] if pay < 24 else 0.0   # conv1 bias (ch = p&1)
        cst[p, 1] = b2[pay >> 1]                        # conv2 bias (oc)
    cst[0:64, 2] = np.asarray(fc1_b, np.float32)
    cst[:, 8] = 1.0

    weights = {"head": (c0, K, b20, b21)}
    nc = build_program(weights)

    in_maps = []
    for c in range(NCORES):
        sl = slice(c * BC, (c + 1) * BC)
        in_maps.append({
            "a_c1": np.ascontiguousarray(a[:, :, sl]),
            "w1": W1, "w2": W2, "wf1": WF1, "wf2": WF2, "cst": cst,
        })
    res = run_bass_kernel_spmd(nc, in_maps, list(range(NCORES)),
                               trace=bool(int(os.environ.get("BASS_TRACE_KERNEL", "0"))))
    if res.exec_time_ns is not None:
        print(f"HW exec time: {res.exec_time_ns} ns")
    global LAST_RESULTS
    LAST_RESULTS = res.results
    out = np.empty((B, 2), dtype=np.float32)
    for c in range(NCORES):
        out[c * BC:(c + 1) * BC] = res.results[c]["y"].T
    return out


# revision 30
# speedup vs baseline: 1.0165x; 1.0165x over previous
"""Trainium2 Bass kernel for nn_BinaryClassifier (CNN + 2-qubit circuit head).

Data-parallel over 8 cores (65536 -> 8192/core), NT=1024 images per tile.

Structure per tile:
  conv1: 12 blocks (4y x 12x out, patch 8x16 = K 128 full), 1 matmul each.
         Bias folded into the PSUM drain (tensor_scalar w/ AP bias).
  pool1: drain -> stage1 max(yp halves, cross-partition-offset TT)
         -> stage2 max+relu (STT) writing 32-row chunks into conv2 K-tiles.
  conv2: 8 out-tiles x 2 accumulated matmuls over [128,N] K-tiles
         (zero weight rows where a row is outside the 6x8 patch).
  pool2: same pattern -> fc1 rhs tiles F2a/F2b.
  fc1:   2 matmuls + Act relu/bias drain.  fc2: 8 small matmuls.
  head:  quantum circuit reduced to 6 cosines, computed once after the loop.
"""
import os, sys
sys.path.insert(0, "/opt/trn_rl_repo")
import numpy as np
import ml_dtypes

from concourse import bass, tile, bacc
from concourse import mybir
from concourse.bass_utils import run_bass_kernel_spmd

dt = mybir.dt
AF = mybir.ActivationFunctionType
ALU = mybir.AluOpType

B = int(os.environ.get("BASS_KERNEL_B", "65536"))
NCORES = int(os.environ.get("BASS_KERNEL_CORES", "8"))
BC = B // NCORES          # images per core
NT = min(1024, BC)        # images per tile
NTILES = BC // NT
NSUB = NT // 128          # fc2 image-subtiles per tile

# conv1 blocking: 12 blocks = 6 by (4 out rows each) x 2 bx (12 out cols each)
# patch per block: 8 rows x 16 cols = 128 K-rows (full)
# M layout: quad (yp, xq) * 32 + payload; payload = pc*4 + pr*2 + ch (24 used)


def _bf16(a):
    return np.asarray(a, dtype=np.float32).astype(np.float16)


# ---------------------------------------------------------------- host packing

def build_a(x):
    """x: [B, 784] -> [12, 128, B] bf16 im2col (block, patch-pixel, image)."""
    n = x.shape[0]
    xb = _bf16(x).reshape(n, 28, 28)
    out = np.empty((12, 128, n), dtype=np.float16)
    for by in range(6):
        for bx in range(2):
            b = by * 2 + bx
            for iy in range(8):
                src = xb[:, 4 * by + iy, 12 * bx:12 * bx + 16]   # [n,16]
                out[b, iy * 16:iy * 16 + 16, :] = src.T
    return out


def _m_decode(m):
    """conv1 M index -> (valid, dy, dx, ch): out pixel local coords."""
    quad, pay = m >> 5, m & 31
    if pay >= 24:
        return False, 0, 0, 0
    yp, xq = quad >> 1, quad & 1
    pc, r = pay >> 2, pay & 3
    pr, ch = r >> 1, r & 1
    return True, 2 * pr + yp, 2 * pc + xq, ch


def build_w1(w1):
    """w1: [2,1,5,5] -> [128, 12*128] bf16 (same lhsT for every block)."""
    W = np.zeros((128, 128), dtype=np.float32)
    w1 = np.asarray(w1, dtype=np.float32).reshape(2, 5, 5)
    for m in range(128):
        ok, dy, dx, ch = _m_decode(m)
        if not ok:
            continue
        for ky in range(5):
            for kx in range(5):
                W[(dy + ky) * 16 + (dx + kx), m] = w1[ch, ky, kx]
    return _bf16(W)


def build_w2(w2):
    """w2: [16,2,5,5] -> [128, 16*128] bf16.

    mm i = j*2 + bxi, j = (a, xh) = 2*a + xh.  K-tile (yh(j), bxi):
    K-row k = 32*slot + pc*4 + pr*2 + ch  (slot = chunk index 0..3)
      chunk by = slot + 2*yh;  pooled R = 2*by + pr, C = 6*bxi + pc.
    M col m = quad(yp2, xp2)*32 + oc*2 + bb:
      out pixel oy = 2*a + yp2, ox = 4*xh + 2*bb + xp2.
    """
    W = np.zeros((128, 16 * 128), dtype=np.float32)
    w2 = np.asarray(w2, dtype=np.float32)
    for a in range(4):
        yh = 0 if a < 2 else 1
        for xh in range(2):
            j = 2 * a + xh
            for bxi in range(2):
                col0 = (j * 2 + bxi) * 128
                for slot in range(4):
                    by = slot + 2 * yh
                    for pc in range(6):
                        for pr in range(2):
                            for ch in range(2):
                                k = 32 * slot + pc * 4 + pr * 2 + ch
                                R = 2 * by + pr
                                C = 6 * bxi + pc
                                for m in range(128):
                                    quad, pay = m >> 5, m & 31
                                    yp2, xp2 = quad >> 1, quad & 1
                                    oc, bb = pay >> 1, pay & 1
                                    oy = 2 * a + yp2
                                    ox = 4 * xh + 2 * bb + xp2
                                    ky, kx = R - oy, C - ox
                                    if 0 <= ky < 5 and 0 <= kx < 5:
                                        W[k, col0 + m] = w2[oc, ch, ky, kx]
    return _bf16(W)


def build_wfc1(fc1_w):
    """fc1_w: [64, 256] -> [128, 2*64] bf16 (2 K-tiles F2a/F2b)."""
    W = np.zeros((128, 2 * 64), dtype=np.float32)
    fc1_w = np.asarray(fc1_w, dtype=np.float32)
    for t in range(2):
        for p in range(128):
            jl, pay = p >> 5, p & 31
            j = 4 * t + jl
            a, xh = j >> 1, j & 1
            oc, bb = pay >> 1, pay & 1
            b_ = 2 * xh + bb
            flat = oc * 16 + a * 4 + b_
            W[p, t * 64:t * 64 + 64] = fc1_w[:, flat]
    return _bf16(W)


def head_constants(qnn_params, fc3_w, fc3_b, fc2_b):
    """Reduce the 2-qubit circuit tail + fc3 to z = c0 + sum Mk*cos(...)."""
    p = np.asarray(qnn_params, dtype=np.float64)

    def ry(t):
        c, s = np.cos(t), np.sin(t)
        return np.array([[c, -s], [s, c]])

    def kron_w0(U):
        return np.kron(U, np.eye(2))

    def kron_w1(U):
        return np.kron(np.eye(2), U)

    CN01 = np.zeros((4, 4)); CN01[0, 0] = CN01[1, 1] = 1; CN01[2, 3] = CN01[3, 2] = 1
    CN10 = np.zeros((4, 4))
    for q0 in range(2):
        for q1 in range(2):
            CN10[((q0 ^ q1) * 2 + q1), q0 * 2 + q1] = 1
    U = np.eye(4)
    U = kron_w0(ry(p[0])) @ U
    U = kron_w1(ry(p[1])) @ U
    U = CN01 @ U
    U = kron_w0(ry(p[2])) @ U
    U = kron_w1(ry(p[3])) @ U
    U = CN10 @ U
    U = kron_w0(ry(p[4])) @ U
    U = kron_w1(ry(p[5])) @ U
    U = CN01 @ U
    U = kron_w0(ry(p[6])) @ U
    U = kron_w1(ry(p[7])) @ U
    S = np.diag([1.0, -1.0, -1.0, 1.0])
    M = 0.25 * (U.T @ S @ U)
    w3 = float(np.asarray(fc3_w).reshape(()))
    b3 = float(np.asarray(fc3_b).reshape(()))
    c0 = float(np.trace(M)) * w3 + b3
    k = {
        "A": 2 * M[0, 3] * w3,   # cos(2x0+2x1)
        "B": 2 * M[1, 2] * w3,   # cos(2x0-2x1)
        "C": 2 * M[0, 2] * w3,   # cos(2x0+2ang)
        "D": 2 * M[1, 3] * w3,   # cos(2x0-2ang)
        "E": 2 * M[0, 1] * w3,   # cos(2x1+2ang)
        "F": 2 * M[2, 3] * w3,   # cos(2x1-2ang)
    }
    return c0, k, float(fc2_b[0]), float(fc2_b[1])


# ---------------------------------------------------------------- bass program

def build_program(weights):
    nc = bacc.Bacc(None, target_bir_lowering=False, debug=False)
    a_d = nc.declare_dram_parameter("a_c1", [12, 128, BC], dt.float16, isOutput=False)
    w1_d = nc.declare_dram_parameter("w1", [128, 128], dt.float16, isOutput=False)
    w2_d = nc.declare_dram_parameter("w2", [128, 16 * 128], dt.float16, isOutput=False)
    wf1_d = nc.declare_dram_parameter("wf1", [128, 2 * 64], dt.float16, isOutput=False)
    wf2_d = nc.declare_dram_parameter("wf2", [64, 2], dt.float16, isOutput=False)
    cst_d = nc.declare_dram_parameter("cst", [128, 16], dt.float32, isOutput=False)
    y_d = nc.declare_dram_parameter("y", [2, BC], dt.float32, isOutput=True)

    c0, K, b20, b21 = weights["head"]
    pi = float(np.pi)

    with tile.TileContext(nc) as tc:
        with tc.tile_pool(name="cw", bufs=1) as cw, \
             tc.tile_pool(name="sx", bufs=3) as sx, \
             tc.tile_pool(name="sc", bufs=3) as sc, \
             tc.tile_pool(name="sm", bufs=2) as sm, \
             tc.tile_pool(name="se", bufs=2) as se, \
             tc.tile_pool(name="sf", bufs=2) as sf, \
             tc.tile_pool(name="hd", bufs=1) as hd, \
             tc.tile_pool(name="p1", bufs=4, space="PSUM") as p1:

            W1 = cw.tile([128, 128], dt.float16)
            nc.sync.dma_start(out=W1[:], in_=w1_d[:])
            W2 = cw.tile([128, 16 * 128], dt.float16)
            nc.sync.dma_start(out=W2[:], in_=w2_d[:])
            WF1 = cw.tile([128, 2 * 64], dt.float16)
            nc.sync.dma_start(out=WF1[:], in_=wf1_d[:])
            WF2 = cw.tile([64, 2], dt.float16)
            nc.sync.dma_start(out=WF2[:], in_=wf2_d[:])
            CST = cw.tile([128, 16], dt.float32)
            nc.sync.dma_start(out=CST[:], in_=cst_d[:])

            # engine rotation: drains on DVE/Act (GpSimd cannot read PSUM),
            # SBUF-only pool stages on DVE/GpSimd
            drain_eng = [nc.scalar]
            tt_eng = [nc.vector, nc.vector]

            xcs = {}

            def issue_input(t):
                xc = sx.tile([128, 12 * NT], dt.float16, tag="xc", name="xc")
                nc.sync.dma_start(
                    out=xc[:].rearrange("p (b n) -> p b n", b=12),
                    in_=a_d[:, :, t * NT:(t + 1) * NT].transpose([1, 0, 2]))
                xcs[t] = xc

            for t in range(min(2, NTILES)):
                issue_input(t)

            for it in range(NTILES):
                n0 = it * NT
                if it + 2 < NTILES:
                    issue_input(it + 2)
                xc = xcs.pop(it)

                # T K-tiles for conv2: [yh] each [128, 2*NT], free = (bx, n)
                T = [se.tile([128, 2 * NT], dt.float16, tag=f"T{yh}",
                             name=f"T{yh}") for yh in range(2)]

                di = 0
                ti = 0
                # conv1: block pairs (by, bx=0/1) share weights -> one matmul
                # of N=2*NT into a 4-bank psum tile
                for by in range(6):
                    psu = [p1.tile([128, NT], dt.float32, tag="ps", name=f"ps{by}{h}")
                           for h in range(2)]
                    for q in range(2 * NT // 512):
                        c0_ = 2 * by * NT + q * 512
                        ph_, qh = q // (NT // 512), q % (NT // 512)
                        nc.tensor.matmul(out=psu[ph_][:, qh * 512:(qh + 1) * 512],
                                         lhsT=W1[:, 0:128],
                                         rhs=xc[:, c0_:c0_ + 512],
                                         start=True, stop=True)
                    # drain + conv1 bias + relu (per-partition AP scalar)
                    C = sc.tile([128, 2 * NT], dt.float16, tag="C")
                    for h in range(2):
                        eng = drain_eng[di % len(drain_eng)]; di += 1
                        if eng is nc.scalar:
                            nc.scalar.activation(out=C[:, h * NT:(h + 1) * NT],
                                                 in_=psu[h][:], func=AF.Relu,
                                                 bias=CST[:, 0:1])
                        else:
                            eng.tensor_scalar(out=C[:, h * NT:(h + 1) * NT],
                                              in0=psu[h][:],
                                              scalar1=CST[:, 0:1], scalar2=0.0,
                                              op0=ALU.add, op1=ALU.max)
                    # stage1: max over yp; upper half copied to base-0 first
                    # (two-input SBUF ops require equal base partitions)
                    S1 = sm.tile([64, 2 * NT], dt.float16, tag="S1")
                    nc.sync.dma_start(out=S1[:], in_=C[64:128, :])
                    M1 = sm.tile([64, 2 * NT], dt.float16, tag="M1")
                    eng = tt_eng[ti % 2]; ti += 1
                    eng.tensor_tensor(out=M1[:], in0=C[0:64, :], in1=S1[:],
                                      op=ALU.max)
                    S2 = sm.tile([32, 2 * NT], dt.float16, tag="S2")
                    nc.sync.dma_start(out=S2[:], in_=M1[32:64, :])
                    # stage2: max over xq + relu -> chunk row of T (both blocks)
                    yh = 0 if by < 4 else 1
                    slot = by - 2 * yh
                    eng = tt_eng[ti % 2]; ti += 1
                    eng.tensor_tensor(
                        out=T[yh][32 * slot:32 * slot + 32, :],
                        in0=M1[0:32, :], in1=S2[:], op=ALU.max)

                # chunks by=2,3 (T0 slots 2,3) also open T1 as slots 0,1
                nc.sync.dma_start(out=T[1][0:64, :], in_=T[0][64:128, :])

                # conv2: 8 out-tiles x 2 accumulated matmuls; pairs (j, j+1)
                # share one 4-bank psum tile and batched drain/stage1
                F2 = [sf.tile([128, NT], dt.float16, tag=f"F2{t}", name=f"F2{t}")
                      for t in range(2)]
                for a_ in range(4):
                    yh = 0 if a_ < 2 else 1
                    ps2 = [p1.tile([128, NT], dt.float32, tag="ps", name=f"p2{a_}{h}")
                           for h in range(2)]
                    for xh in range(2):
                        j = 2 * a_ + xh
                        for bxi in range(2):
                            for q in range(NT // 512):
                                nc.tensor.matmul(
                                    out=ps2[xh][:, q * 512:(q + 1) * 512],
                                    lhsT=W2[:, (j * 2 + bxi) * 128:(j * 2 + bxi + 1) * 128],
                                    rhs=T[yh][:, bxi * NT + q * 512:bxi * NT + (q + 1) * 512],
                                    start=(bxi == 0), stop=(bxi == 1))
                    D = sc.tile([128, 2 * NT], dt.float16, tag="D")
                    for h in range(2):
                        eng = drain_eng[di % len(drain_eng)]; di += 1
                        if eng is nc.scalar:
                            nc.scalar.activation(out=D[:, h * NT:(h + 1) * NT],
                                                 in_=ps2[h][:], func=AF.Relu,
                                                 bias=CST[:, 1:2])
                        else:
                            eng.tensor_scalar(out=D[:, h * NT:(h + 1) * NT],
                                              in0=ps2[h][:],
                                              scalar1=CST[:, 1:2], scalar2=0.0,
                                              op0=ALU.add, op1=ALU.max)
                    S1b = sm.tile([64, 2 * NT], dt.float16, tag="S1b")
                    nc.sync.dma_start(out=S1b[:], in_=D[64:128, :])
                    M2 = sm.tile([64, 2 * NT], dt.float16, tag="M2")
                    eng = tt_eng[ti % 2]; ti += 1
                    eng.tensor_tensor(out=M2[:], in0=D[0:64, :], in1=S1b[:],
                                      op=ALU.max)
                    S2b = sm.tile([32, 2 * NT], dt.float16, tag="S2b")
                    nc.sync.dma_start(out=S2b[:], in_=M2[32:64, :])
                    for xh in range(2):
                        j = 2 * a_ + xh
                        ft, jl = j // 4, j % 4
                        eng = tt_eng[ti % 2]; ti += 1
                        eng.tensor_tensor(
                            out=F2[ft][32 * jl:32 * jl + 32, :],
                            in0=M2[0:32, xh * NT:(xh + 1) * NT],
                            in1=S2b[:, xh * NT:(xh + 1) * NT], op=ALU.max)

                # fc1 (K=256 over 2 tiles) -> relu -> F1 bf16
                psf = p1.tile([64, NT], dt.float32, tag="ps")
                for t_ in range(2):
                    for q in range(NT // 512):
                        nc.tensor.matmul(out=psf[:, q * 512:(q + 1) * 512],
                                         lhsT=WF1[:, 64 * t_:64 * (t_ + 1)],
                                         rhs=F2[t_][:, q * 512:(q + 1) * 512],
                                         start=(t_ == 0), stop=(t_ == 1))
                F1 = sf.tile([64, NT], dt.float16, tag="F1")
                nc.scalar.activation(out=F1[:], in_=psf[:, 0:NT], func=AF.Relu,
                                     bias=CST[0:64, 2:3])

                # fc2 img-major: NSUB matmuls N=2 -> psum [128, 2*NSUB]
                psg = p1.tile([128, NT], dt.float32, tag="ps")
                for s in range(NSUB):
                    nc.tensor.matmul(out=psg[:, 2 * s:2 * s + 2],
                                     lhsT=F1[:, 128 * s:128 * (s + 1)],
                                     rhs=WF2[:], start=True, stop=True)
                H = hd.tile([128, 2 * NSUB], dt.float32, tag="H", bufs=2)
                nc.vector.tensor_copy(out=H[:], in_=psg[:, 0:2 * NSUB])

                # ---- head (per tile): H [128, (s, c)] -> y [128, NSUB]
                NC_ = NSUB
                Hv = H[:].rearrange("p (u c) -> p u c", c=2)
                x0 = Hv[:, :, 0]
                x1 = Hv[:, :, 1]
                t0 = hd.tile([128, NC_], dt.float32, tag="t0", bufs=2)
                nc.vector.tensor_scalar(out=t0[:], in0=x0, scalar1=-1.0,
                                        scalar2=pi - b20, op0=ALU.mult, op1=ALU.add)
                t1 = hd.tile([128, NC_], dt.float32, tag="t1", bufs=2)
                nc.vector.tensor_scalar(out=t1[:], in0=x1, scalar1=-1.0,
                                        scalar2=pi - b21, op0=ALU.mult, op1=ALU.add)
                ang = hd.tile([128, NC_], dt.float32, tag="ang", bufs=2)
                nc.vector.tensor_tensor(out=ang[:], in0=t0[:], in1=t1[:], op=ALU.mult)

                qpi = pi / 4
                hb = {"A": b20 + b21 + qpi, "B": b20 - b21 + qpi,
                      "C": b20 + qpi, "D": b20 + qpi,
                      "E": b21 + qpi, "F": b21 + qpi}
                AR = hd.tile([128, 6 * NC_], dt.float32, tag="AR", bufs=2)
                plan = (("A", x0, x1, ALU.add), ("B", x0, x1, ALU.subtract),
                        ("C", x0, ang[:], ALU.add), ("D", x0, ang[:], ALU.subtract),
                        ("E", x1, ang[:], ALU.add), ("F", x1, ang[:], ALU.subtract))
                for i, (nm, a0, a1, op) in enumerate(plan):
                    nc.vector.scalar_tensor_tensor(
                        out=AR[:, NC_ * i:NC_ * (i + 1)], in0=a0, scalar=hb[nm],
                        in1=a1, op0=ALU.add, op1=op)
                tq = hd.tile([128, 6 * NC_], dt.float32, tag="tq", bufs=2)
                nc.vector.tensor_scalar(out=tq[:], in0=AR[:], scalar1=float(1 / pi),
                                        scalar2=None, op0=ALU.mult)
                ti_ = hd.tile([128, 6 * NC_], dt.int32, tag="ti", bufs=2)
                nc.vector.tensor_copy(out=ti_[:], in_=tq[:])
                tf_ = hd.tile([128, 6 * NC_], dt.float32, tag="tf", bufs=2)
                nc.vector.tensor_copy(out=tf_[:], in_=ti_[:])
                hh = hd.tile([128, 6 * NC_], dt.float32, tag="hh", bufs=2)
                nc.vector.scalar_tensor_tensor(out=hh[:], in0=tf_[:], scalar=-pi,
                                               in1=AR[:], op0=ALU.mult, op1=ALU.add)
                SN = hd.tile([128, 6 * NC_], dt.float32, tag="SN", bufs=2)
                nc.scalar.activation(out=SN[:], in_=hh[:], func=AF.Sin, scale=2.0)
                cosv = {nm: SN[:, NC_ * i:NC_ * (i + 1)]
                        for i, nm in enumerate("ABCDEF")}

                acc = hd.tile([128, NC_], dt.float32, tag="acc0", bufs=2)
                nc.vector.tensor_scalar(out=acc[:], in0=cosv["A"], scalar1=K["A"],
                                        scalar2=c0, op0=ALU.mult, op1=ALU.add)
                for i, nm in enumerate("BCDE"):
                    acc2 = hd.tile([128, NC_], dt.float32, tag=f"acc{i+1}", bufs=2)
                    nc.vector.scalar_tensor_tensor(out=acc2[:], in0=cosv[nm][:],
                                                   scalar=K[nm], in1=acc[:],
                                                   op0=ALU.mult, op1=ALU.add)
                    acc = acc2
                Yt = hd.tile([128, NC_], dt.float32, tag="Yt", bufs=2)
                nc.vector.scalar_tensor_tensor(out=Yt[:], in0=cosv["F"],
                                               scalar=K["F"], in1=acc[:],
                                               op0=ALU.mult, op1=ALU.add)

                # final: out0=-ln(1+e^{1-2y}), out1=-ln(1+e^{2y-1})
                V = hd.tile([128, NC_], dt.float32, tag="V", bufs=2)
                nc.scalar.activation(out=V[:], in_=Yt[:], func=AF.Exp,
                                     bias=CST[:, 8:9], scale=-2.0)
                Wr = hd.tile([128, NC_], dt.float32, tag="Wr", bufs=2)
                nc.vector.reciprocal(out=Wr[:], in_=V[:])
                L0 = hd.tile([128, NC_], dt.float32, tag="L0", bufs=2)
                nc.scalar.activation(out=L0[:], in_=V[:], func=AF.Ln,
                                     bias=CST[:, 8:9], scale=1.0)
                L1 = hd.tile([128, NC_], dt.float32, tag="L1", bufs=2)
                nc.scalar.activation(out=L1[:], in_=Wr[:], func=AF.Ln,
                                     bias=CST[:, 8:9], scale=1.0)
                O = hd.tile([128, 2 * NC_], dt.float32, tag="O", bufs=2)
                Ov = O[:].rearrange("p (c u) -> p c u", c=2)
                nc.vector.tensor_scalar(out=Ov[:, 0, :], in0=L0[:], scalar1=-1.0,
                                        scalar2=None, op0=ALU.mult)
                nc.vector.tensor_scalar(out=Ov[:, 1, :], in0=L1[:], scalar1=-1.0,
                                        scalar2=None, op0=ALU.mult)
                for c in range(2):
                    nc.sync.dma_start(
                        out=y_d[c, n0:n0 + NT].rearrange("(s p) -> p s", p=128),
                        in_=Ov[:, c, :].rearrange("p s -> p s"))

    nc.compile()
    return nc


def kernel(x, conv1_w, conv1_b, conv2_w, conv2_b, fc1_w, fc1_b,
           fc2_w, fc2_b, fc3_w, fc3_b, qnn_params):
    x = np.asarray(x, dtype=np.float32).reshape(B, 784)
    a = build_a(x)
    W1 = build_w1(conv1_w)
    W2 = build_w2(conv2_w)
    WF1 = build_wfc1(fc1_w)
    WF2 = _bf16(np.asarray(fc2_w, np.float32).T)  # [64, 2]
    c0, K, b20, b21 = head_constants(qnn_params, fc3_w, fc3_b,
                                     np.asarray(fc2_b, np.float32))
    cst = np.zeros((128, 16), dtype=np.float32)
    b1 = np.asarray(conv1_b, np.float32)
    b2 = np.asarray(conv2_b, np.float32)
    for p in range(128):
        pay = p & 31
        cst[p, 0] = b1[pay & 1] if pay < 24 else 0.0   # conv1 bias (ch = p&1)
        cst[p, 1] = b2[pay >> 1]                        # conv2 bias (oc)
    cst[0:64, 2] = np.asarray(fc1_b, np.float32)
    cst[:, 8] = 1.0

    weights = {"head": (c0, K, b20, b21)}
    nc = build_program(weights)

    in_maps = []
    for c in range(NCORES):
        sl = slice(c * BC, (c + 1) * BC)
        in_maps.append({
            "a_c1": np.ascontiguousarray(a[:, :, sl]),
            "w1": W1, "w2": W2, "wf1": WF1, "wf2": WF2, "cst": cst,
        })
    res = run_bass_kernel_spmd(nc, in_maps, list(range(NCORES)),
                               trace=bool(int(os.environ.get("BASS_TRACE_KERNEL", "0"))))
    if res.exec_time_ns is not None:
        print(f"HW exec time: {res.exec_time_ns} ns")
    global LAST_RESULTS
    LAST_RESULTS = res.results
    out = np.empty((B, 2), dtype=np.float32)
    for c in range(NCORES):
        out[c * BC:(c + 1) * BC] = res.results[c]["y"].T
    return out


# revision 32
# speedup vs baseline: 1.3123x; 1.2909x over previous
"""Trainium2 Bass kernel for nn_BinaryClassifier (CNN + 2-qubit circuit head).

Data-parallel over 8 cores (65536 -> 8192/core), NT=1024 images per tile.

Structure per tile:
  conv1: 12 blocks (4y x 12x out, patch 8x16 = K 128 full), 1 matmul each.
         Bias folded into the PSUM drain (tensor_scalar w/ AP bias).
  pool1: drain -> stage1 max(yp halves, cross-partition-offset TT)
         -> stage2 max+relu (STT) writing 32-row chunks into conv2 K-tiles.
  conv2: 8 out-tiles x 2 accumulated matmuls over [128,N] K-tiles
         (zero weight rows where a row is outside the 6x8 patch).
  pool2: same pattern -> fc1 rhs tiles F2a/F2b.
  fc1:   2 matmuls + Act relu/bias drain.  fc2: 8 small matmuls.
  head:  quantum circuit reduced to 6 cosines, computed once after the loop.
"""
import os, sys
sys.path.insert(0, "/opt/trn_rl_repo")
import numpy as np
import ml_dtypes

from concourse import bass, tile, bacc
from concourse import mybir
from concourse.bass_utils import run_bass_kernel_spmd

dt = mybir.dt
AF = mybir.ActivationFunctionType
ALU = mybir.AluOpType

B = int(os.environ.get("BASS_KERNEL_B", "65536"))
NCORES = int(os.environ.get("BASS_KERNEL_CORES", "8"))
BC = B // NCORES          # images per core
NT = min(1024, BC)        # images per tile
NTILES = BC // NT
NSUB = NT // 128          # fc2 image-subtiles per tile

# conv1 blocking: 12 blocks = 6 by (4 out rows each) x 2 bx (12 out cols each)
# patch per block: 8 rows x 16 cols = 128 K-rows (full)
# M layout: quad (yp, xq) * 32 + payload; payload = pc*4 + pr*2 + ch (24 used)


def _bf16(a):
    return np.asarray(a, dtype=np.float32).astype(np.float16)


# ---------------------------------------------------------------- host packing

def build_a(x):
    """x: [B, 784] -> [12, 128, B] bf16 im2col (block, patch-pixel, image)."""
    n = x.shape[0]
    xb = _bf16(x).reshape(n, 28, 28)
    out = np.empty((12, 128, n), dtype=np.float16)
    for by in range(6):
        for bx in range(2):
            b = by * 2 + bx
            for iy in range(8):
                src = xb[:, 4 * by + iy, 12 * bx:12 * bx + 16]   # [n,16]
                out[b, iy * 16:iy * 16 + 16, :] = src.T
    return out


def _m_decode(m):
    """conv1 M index -> (valid, dy, dx, ch): out pixel local coords."""
    quad, pay = m >> 5, m & 31
    if pay >= 24:
        return False, 0, 0, 0
    yp, xq = quad >> 1, quad & 1
    pc, r = pay >> 2, pay & 3
    pr, ch = r >> 1, r & 1
    return True, 2 * pr + yp, 2 * pc + xq, ch


def build_w1(w1):
    """w1: [2,1,5,5] -> [128, 12*128] bf16 (same lhsT for every block)."""
    W = np.zeros((128, 128), dtype=np.float32)
    w1 = np.asarray(w1, dtype=np.float32).reshape(2, 5, 5)
    for m in range(128):
        ok, dy, dx, ch = _m_decode(m)
        if not ok:
            continue
        for ky in range(5):
            for kx in range(5):
                W[(dy + ky) * 16 + (dx + kx), m] = w1[ch, ky, kx]
    return _bf16(W)


def build_w2(w2):
    """w2: [16,2,5,5] -> [128, 16*128] bf16.

    mm i = j*2 + bxi, j = (a, xh) = 2*a + xh.  K-tile (yh(j), bxi):
    K-row k = 32*slot + pc*4 + pr*2 + ch  (slot = chunk index 0..3)
      chunk by = slot + 2*yh;  pooled R = 2*by + pr, C = 6*bxi + pc.
    M col m = quad(yp2, xp2)*32 + oc*2 + bb:
      out pixel oy = 2*a + yp2, ox = 4*xh + 2*bb + xp2.
    """
    W = np.zeros((128, 16 * 128), dtype=np.float32)
    w2 = np.asarray(w2, dtype=np.float32)
    for a in range(4):
        yh = 0 if a < 2 else 1
        for xh in range(2):
            j = 2 * a + xh
            for bxi in range(2):
                col0 = (j * 2 + bxi) * 128
                for slot in range(4):
                    by = slot + 2 * yh
                    for pc in range(6):
                        for pr in range(2):
                            for ch in range(2):
                                k = 32 * slot + pc * 4 + pr * 2 + ch
                                R = 2 * by + pr
                                C = 6 * bxi + pc
                                for m in range(128):
                                    quad, pay = m >> 5, m & 31
                                    yp2, xp2 = quad >> 1, quad & 1
                                    oc, bb = pay >> 1, pay & 1
                                    oy = 2 * a + yp2
                                    ox = 4 * xh + 2 * bb + xp2
                                    ky, kx = R - oy, C - ox
                                    if 0 <= ky < 5 and 0 <= kx < 5:
                                        W[k, col0 + m] = w2[oc, ch, ky, kx]
    return _bf16(W)


def build_wfc1(fc1_w):
    """fc1_w: [64, 256] -> [128, 2*64] bf16 (2 K-tiles F2a/F2b)."""
    W = np.zeros((128, 2 * 64), dtype=np.float32)
    fc1_w = np.asarray(fc1_w, dtype=np.float32)
    for t in range(2):
        for p in range(128):
            jl, pay = p >> 5, p & 31
            j = 4 * t + jl
            a, xh = j >> 1, j & 1
            oc, bb = pay >> 1, pay & 1
            b_ = 2 * xh + bb
            flat = oc * 16 + a * 4 + b_
            W[p, t * 64:t * 64 + 64] = fc1_w[:, flat]
    return _bf16(W)


def head_constants(qnn_params, fc3_w, fc3_b, fc2_b):
    """Reduce the 2-qubit circuit tail + fc3 to z = c0 + sum Mk*cos(...)."""
    p = np.asarray(qnn_params, dtype=np.float64)

    def ry(t):
        c, s = np.cos(t), np.sin(t)
        return np.array([[c, -s], [s, c]])

    def kron_w0(U):
        return np.kron(U, np.eye(2))

    def kron_w1(U):
        return np.kron(np.eye(2), U)

    CN01 = np.zeros((4, 4)); CN01[0, 0] = CN01[1, 1] = 1; CN01[2, 3] = CN01[3, 2] = 1
    CN10 = np.zeros((4, 4))
    for q0 in range(2):
        for q1 in range(2):
            CN10[((q0 ^ q1) * 2 + q1), q0 * 2 + q1] = 1
    U = np.eye(4)
    U = kron_w0(ry(p[0])) @ U
    U = kron_w1(ry(p[1])) @ U
    U = CN01 @ U
    U = kron_w0(ry(p[2])) @ U
    U = kron_w1(ry(p[3])) @ U
    U = CN10 @ U
    U = kron_w0(ry(p[4])) @ U
    U = kron_w1(ry(p[5])) @ U
    U = CN01 @ U
    U = kron_w0(ry(p[6])) @ U
    U = kron_w1(ry(p[7])) @ U
    S = np.diag([1.0, -1.0, -1.0, 1.0])
    M = 0.25 * (U.T @ S @ U)
    w3 = float(np.asarray(fc3_w).reshape(()))
    b3 = float(np.asarray(fc3_b).reshape(()))
    c0 = float(np.trace(M)) * w3 + b3
    k = {
        "A": 2 * M[0, 3] * w3,   # cos(2x0+2x1)
        "B": 2 * M[1, 2] * w3,   # cos(2x0-2x1)
        "C": 2 * M[0, 2] * w3,   # cos(2x0+2ang)
        "D": 2 * M[1, 3] * w3,   # cos(2x0-2ang)
        "E": 2 * M[0, 1] * w3,   # cos(2x1+2ang)
        "F": 2 * M[2, 3] * w3,   # cos(2x1-2ang)
    }
    return c0, k, float(fc2_b[0]), float(fc2_b[1])


# ---------------------------------------------------------------- bass program

def build_program(weights):
    nc = bacc.Bacc(None, target_bir_lowering=False, debug=False)
    a_d = nc.declare_dram_parameter("a_c1", [12, 128, BC], dt.float16, isOutput=False)
    w1_d = nc.declare_dram_parameter("w1", [128, 128], dt.float16, isOutput=False)
    w2_d = nc.declare_dram_parameter("w2", [128, 16 * 128], dt.float16, isOutput=False)
    wf1_d = nc.declare_dram_parameter("wf1", [128, 2 * 64], dt.float16, isOutput=False)
    wf2_d = nc.declare_dram_parameter("wf2", [64, 2], dt.float16, isOutput=False)
    cst_d = nc.declare_dram_parameter("cst", [128, 16], dt.float32, isOutput=False)
    y_d = nc.declare_dram_parameter("y", [2, BC], dt.float32, isOutput=True)

    c0, K, b20, b21 = weights["head"]
    pi = float(np.pi)

    with tile.TileContext(nc) as tc:
        with tc.tile_pool(name="cw", bufs=1) as cw, \
             tc.tile_pool(name="sx", bufs=3) as sx, \
             tc.tile_pool(name="sc", bufs=3) as sc, \
             tc.tile_pool(name="sm", bufs=2) as sm, \
             tc.tile_pool(name="se", bufs=2) as se, \
             tc.tile_pool(name="sf", bufs=2) as sf, \
             tc.tile_pool(name="hd", bufs=1) as hd, \
             tc.tile_pool(name="p1", bufs=4, space="PSUM") as p1:

            W1 = cw.tile([128, 128], dt.float16)
            nc.sync.dma_start(out=W1[:], in_=w1_d[:])
            W2 = cw.tile([128, 16 * 128], dt.float16)
            nc.sync.dma_start(out=W2[:], in_=w2_d[:])
            WF1 = cw.tile([128, 2 * 64], dt.float16)
            nc.sync.dma_start(out=WF1[:], in_=wf1_d[:])
            WF2 = cw.tile([64, 2], dt.float16)
            nc.sync.dma_start(out=WF2[:], in_=wf2_d[:])
            CST = cw.tile([128, 16], dt.float32)
            nc.sync.dma_start(out=CST[:], in_=cst_d[:])
            Hall = cw.tile([128, NTILES * 2 * NSUB], dt.float32)

            # engine rotation: drains on DVE/Act (GpSimd cannot read PSUM),
            # SBUF-only pool stages on DVE/GpSimd
            drain_eng = [nc.scalar]
            tt_eng = [nc.vector, nc.vector]

            xcs = {}

            def issue_input(t):
                xc = sx.tile([128, 12 * NT], dt.float16, tag="xc", name="xc")
                nc.sync.dma_start(
                    out=xc[:].rearrange("p (b n) -> p b n", b=12),
                    in_=a_d[:, :, t * NT:(t + 1) * NT].transpose([1, 0, 2]))
                xcs[t] = xc

            for t in range(min(2, NTILES)):
                issue_input(t)

            for it in range(NTILES):
                n0 = it * NT
                if it + 2 < NTILES:
                    issue_input(it + 2)
                xc = xcs.pop(it)

                # T K-tiles for conv2: [yh] each [128, 2*NT], free = (bx, n)
                T = [se.tile([128, 2 * NT], dt.float16, tag=f"T{yh}",
                             name=f"T{yh}") for yh in range(2)]

                di = 0
                ti = 0
                # conv1: block pairs (by, bx=0/1) share weights -> one matmul
                # of N=2*NT into a 4-bank psum tile
                for by in range(6):
                    psu = [p1.tile([128, NT], dt.float32, tag="ps", name=f"ps{by}{h}")
                           for h in range(2)]
                    for q in range(2 * NT // 512):
                        c0_ = 2 * by * NT + q * 512
                        ph_, qh = q // (NT // 512), q % (NT // 512)
                        nc.tensor.matmul(out=psu[ph_][:, qh * 512:(qh + 1) * 512],
                                         lhsT=W1[:, 0:128],
                                         rhs=xc[:, c0_:c0_ + 512],
                                         start=True, stop=True)
                    # drain + conv1 bias + relu (per-partition AP scalar)
                    C = sc.tile([128, 2 * NT], dt.float16, tag="C")
                    for h in range(2):
                        eng = drain_eng[di % len(drain_eng)]; di += 1
                        if eng is nc.scalar:
                            nc.scalar.activation(out=C[:, h * NT:(h + 1) * NT],
                                                 in_=psu[h][:], func=AF.Relu,
                                                 bias=CST[:, 0:1])
                        else:
                            eng.tensor_scalar(out=C[:, h * NT:(h + 1) * NT],
                                              in0=psu[h][:],
                                              scalar1=CST[:, 0:1], scalar2=0.0,
                                              op0=ALU.add, op1=ALU.max)
                    # stage1: max over yp; upper half copied to base-0 first
                    # (two-input SBUF ops require equal base partitions)
                    S1 = sm.tile([64, 2 * NT], dt.float16, tag="S1")
                    nc.sync.dma_start(out=S1[:], in_=C[64:128, :])
                    M1 = sm.tile([64, 2 * NT], dt.float16, tag="M1")
                    eng = tt_eng[ti % 2]; ti += 1
                    eng.tensor_tensor(out=M1[:], in0=C[0:64, :], in1=S1[:],
                                      op=ALU.max)
                    S2 = sm.tile([32, 2 * NT], dt.float16, tag="S2")
                    nc.sync.dma_start(out=S2[:], in_=M1[32:64, :])
                    # stage2: max over xq + relu -> chunk row of T (both blocks)
                    yh = 0 if by < 4 else 1
                    slot = by - 2 * yh
                    eng = tt_eng[ti % 2]; ti += 1
                    eng.tensor_tensor(
                        out=T[yh][32 * slot:32 * slot + 32, :],
                        in0=M1[0:32, :], in1=S2[:], op=ALU.max)

                # chunks by=2,3 (T0 slots 2,3) also open T1 as slots 0,1
                nc.sync.dma_start(out=T[1][0:64, :], in_=T[0][64:128, :])

                # conv2: 8 out-tiles x 2 accumulated matmuls; pairs (j, j+1)
                # share one 4-bank psum tile and batched drain/stage1
                F2 = [sf.tile([128, NT], dt.float16, tag=f"F2{t}", name=f"F2{t}")
                      for t in range(2)]
                for a_ in range(4):
                    yh = 0 if a_ < 2 else 1
                    ps2 = [p1.tile([128, NT], dt.float32, tag="ps", name=f"p2{a_}{h}")
                           for h in range(2)]
                    for xh in range(2):
                        j = 2 * a_ + xh
                        for bxi in range(2):
                            for q in range(NT // 512):
                                nc.tensor.matmul(
                                    out=ps2[xh][:, q * 512:(q + 1) * 512],
                                    lhsT=W2[:, (j * 2 + bxi) * 128:(j * 2 + bxi + 1) * 128],
                                    rhs=T[yh][:, bxi * NT + q * 512:bxi * NT + (q + 1) * 512],
                                    start=(bxi == 0), stop=(bxi == 1))
                    D = sc.tile([128, 2 * NT], dt.float16, tag="D")
                    for h in range(2):
                        eng = drain_eng[di % len(drain_eng)]; di += 1
                        if eng is nc.scalar:
                            nc.scalar.activation(out=D[:, h * NT:(h + 1) * NT],
                                                 in_=ps2[h][:], func=AF.Relu,
                                                 bias=CST[:, 1:2])
                        else:
                            eng.tensor_scalar(out=D[:, h * NT:(h + 1) * NT],
                                              in0=ps2[h][:],
                                              scalar1=CST[:, 1:2], scalar2=0.0,
                                              op0=ALU.add, op1=ALU.max)
                    S1b = sm.tile([64, 2 * NT], dt.float16, tag="S1b")
                    nc.sync.dma_start(out=S1b[:], in_=D[64:128, :])
                    M2 = sm.tile([64, 2 * NT], dt.float16, tag="M2")
                    eng = tt_eng[ti % 2]; ti += 1
                    eng.tensor_tensor(out=M2[:], in0=D[0:64, :], in1=S1b[:],
                                      op=ALU.max)
                    S2b = sm.tile([32, 2 * NT], dt.float16, tag="S2b")
                    nc.sync.dma_start(out=S2b[:], in_=M2[32:64, :])
                    for xh in range(2):
                        j = 2 * a_ + xh
                        ft, jl = j // 4, j % 4
                        eng = tt_eng[ti % 2]; ti += 1
                        eng.tensor_tensor(
                            out=F2[ft][32 * jl:32 * jl + 32, :],
                            in0=M2[0:32, xh * NT:(xh + 1) * NT],
                            in1=S2b[:, xh * NT:(xh + 1) * NT], op=ALU.max)

                # fc1 (K=256 over 2 tiles) -> relu -> F1 bf16
                psf = p1.tile([64, NT], dt.float32, tag="ps")
                for t_ in range(2):
                    for q in range(NT // 512):
                        nc.tensor.matmul(out=psf[:, q * 512:(q + 1) * 512],
                                         lhsT=WF1[:, 64 * t_:64 * (t_ + 1)],
                                         rhs=F2[t_][:, q * 512:(q + 1) * 512],
                                         start=(t_ == 0), stop=(t_ == 1))
                F1 = sf.tile([64, NT], dt.float16, tag="F1")
                nc.scalar.activation(out=F1[:], in_=psf[:, 0:NT], func=AF.Relu,
                                     bias=CST[0:64, 2:3])

                # fc2 img-major: NSUB matmuls N=2 -> psum [128, 2*NSUB]
                psg = p1.tile([128, NT], dt.float32, tag="ps")
                for s in range(NSUB):
                    nc.tensor.matmul(out=psg[:, 2 * s:2 * s + 2],
                                     lhsT=F1[:, 128 * s:128 * (s + 1)],
                                     rhs=WF2[:], start=True, stop=True)
                nc.vector.tensor_copy(
                    out=Hall[:, it * 2 * NSUB:(it + 1) * 2 * NSUB],
                    in_=psg[:, 0:2 * NSUB])


            # ---- head once: Hall [128, (t, s, c)] -> Yall [128, (t, s)]
            NC_ = NTILES * NSUB
            Hv = Hall[:].rearrange("p (u c) -> p u c", c=2)
            x0 = Hv[:, :, 0]
            x1 = Hv[:, :, 1]
            t0 = hd.tile([128, NC_], dt.float32, tag="t0")
            nc.vector.tensor_scalar(out=t0[:], in0=x0, scalar1=-1.0,
                                    scalar2=pi - b20, op0=ALU.mult, op1=ALU.add)
            t1 = hd.tile([128, NC_], dt.float32, tag="t1")
            nc.vector.tensor_scalar(out=t1[:], in0=x1, scalar1=-1.0,
                                    scalar2=pi - b21, op0=ALU.mult, op1=ALU.add)
            ang = hd.tile([128, NC_], dt.float32, tag="ang")
            nc.vector.tensor_tensor(out=ang[:], in0=t0[:], in1=t1[:], op=ALU.mult)

            qpi = pi / 4
            hb = {"A": b20 + b21 + qpi, "B": b20 - b21 + qpi,
                  "C": b20 + qpi, "D": b20 + qpi,
                  "E": b21 + qpi, "F": b21 + qpi}
            AR = hd.tile([128, 6 * NC_], dt.float32, tag="AR")
            plan = (("A", x0, x1, ALU.add), ("B", x0, x1, ALU.subtract),
                    ("C", x0, ang[:], ALU.add), ("D", x0, ang[:], ALU.subtract),
                    ("E", x1, ang[:], ALU.add), ("F", x1, ang[:], ALU.subtract))
            for i, (nm, a0, a1, op) in enumerate(plan):
                nc.vector.scalar_tensor_tensor(
                    out=AR[:, NC_ * i:NC_ * (i + 1)], in0=a0, scalar=hb[nm],
                    in1=a1, op0=ALU.add, op1=op)
            tq = hd.tile([128, 6 * NC_], dt.float32, tag="tq")
            nc.vector.tensor_scalar(out=tq[:], in0=AR[:], scalar1=float(1 / pi),
                                    scalar2=None, op0=ALU.mult)
            ti_ = hd.tile([128, 6 * NC_], dt.int32, tag="ti")
            nc.vector.tensor_copy(out=ti_[:], in_=tq[:])
            tf_ = hd.tile([128, 6 * NC_], dt.float32, tag="tf")
            nc.vector.tensor_copy(out=tf_[:], in_=ti_[:])
            hh = hd.tile([128, 6 * NC_], dt.float32, tag="hh")
            nc.vector.scalar_tensor_tensor(out=hh[:], in0=tf_[:], scalar=-pi,
                                           in1=AR[:], op0=ALU.mult, op1=ALU.add)
            SN = hd.tile([128, 6 * NC_], dt.float32, tag="SN")
            nc.scalar.activation(out=SN[:], in_=hh[:], func=AF.Sin, scale=2.0)
            cosv = {nm: SN[:, NC_ * i:NC_ * (i + 1)]
                    for i, nm in enumerate("ABCDEF")}

            acc = hd.tile([128, NC_], dt.float32, tag="acc0")
            nc.vector.tensor_scalar(out=acc[:], in0=cosv["A"], scalar1=K["A"],
                                    scalar2=c0, op0=ALU.mult, op1=ALU.add)
            for i, nm in enumerate("BCDE"):
                acc2 = hd.tile([128, NC_], dt.float32, tag=f"acc{i+1}")
                nc.vector.scalar_tensor_tensor(out=acc2[:], in0=cosv[nm][:],
                                               scalar=K[nm], in1=acc[:],
                                               op0=ALU.mult, op1=ALU.add)
                acc = acc2
            Yall = hd.tile([128, NC_], dt.float32, tag="Yall")
            nc.vector.scalar_tensor_tensor(out=Yall[:], in0=cosv["F"],
                                           scalar=K["F"], in1=acc[:],
                                           op0=ALU.mult, op1=ALU.add)

            V = hd.tile([128, NC_], dt.float32, tag="V")
            nc.scalar.activation(out=V[:], in_=Yall[:], func=AF.Exp,
                                 bias=CST[:, 8:9], scale=-2.0)
            Wr = hd.tile([128, NC_], dt.float32, tag="Wr")
            nc.vector.reciprocal(out=Wr[:], in_=V[:])
            L0 = hd.tile([128, NC_], dt.float32, tag="L0")
            nc.scalar.activation(out=L0[:], in_=V[:], func=AF.Ln,
                                 bias=CST[:, 8:9], scale=1.0)
            L1 = hd.tile([128, NC_], dt.float32, tag="L1")
            nc.scalar.activation(out=L1[:], in_=Wr[:], func=AF.Ln,
                                 bias=CST[:, 8:9], scale=1.0)
            O = hd.tile([128, 2 * NC_], dt.float32, tag="O")
            Ov = O[:].rearrange("p (c u) -> p c u", c=2)
            nc.vector.tensor_scalar(out=Ov[:, 0, :], in0=L0[:], scalar1=-1.0,
                                    scalar2=None, op0=ALU.mult)
            nc.vector.tensor_scalar(out=Ov[:, 1, :], in0=L1[:], scalar1=-1.0,
                                    scalar2=None, op0=ALU.mult)
            for c in range(2):
                nc.sync.dma_start(
                    out=y_d[c, :].rearrange("(t s p) -> p t s", p=128, s=NSUB),
                    in_=Ov[:, c, :].rearrange("p (t s) -> p t s", s=NSUB))

    nc.compile()
    return nc


def kernel(x, conv1_w, conv1_b, conv2_w, conv2_b, fc1_w, fc1_b,
           fc2_w, fc2_b, fc3_w, fc3_b, qnn_params):
    x = np.asarray(x, dtype=np.float32).reshape(B, 784)
    a = build_a(x)
    W1 = build_w1(conv1_w)
    W2 = build_w2(conv2_w)
    WF1 = build_wfc1(fc1_w)
    WF2 = _bf16(np.asarray(fc2_w, np.float32).T)  # [64, 2]
    c0, K, b20, b21 = head_constants(qnn_params, fc3_w, fc3_b,
                                     np.asarray(fc2_b, np.float32))
    cst = np.zeros((128, 16), dtype=np.float32)
    b1 = np.asarray(conv1_b, np.float32)
    b2 = np.asarray(conv2_b, np.float32)
    for p in range(128):
        pay = p & 31
        cst[p, 0] = b1[pay & 1] if pay < 24 else 0.0   # conv1 bias (ch = p&1)
        cst[p, 1] = b2[pay >> 1]                        # conv2 bias (oc)
    cst[0:64, 2] = np.asarray(fc1_b, np.float32)
    cst[:, 8] = 1.0

    weights = {"head": (c0, K, b20, b21)}
    nc = build_program(weights)

    in_maps = []
    for c in range(NCORES):
        sl = slice(c * BC, (c + 1) * BC)
        in_maps.append({
            "a_c1": np.ascontiguousarray(a[:, :, sl]),
            "w1": W1, "w2": W2, "wf1": WF1, "wf2": WF2, "cst": cst,
        })
    res = run_bass_kernel_spmd(nc, in_maps, list(range(NCORES)),
                               trace=bool(int(os.environ.get("BASS_TRACE_KERNEL", "0"))))
    if res.exec_time_ns is not None:
        print(f"HW exec time: {res.exec_time_ns} ns")
    global LAST_RESULTS
    LAST_RESULTS = res.results
    out = np.empty((B, 2), dtype=np.float32)
    for c in range(NCORES):
        out[c * BC:(c + 1) * BC] = res.results[c]["y"].T
    return out


# revision 33
# speedup vs baseline: 1.3889x; 1.0584x over previous
"""Trainium2 Bass kernel for nn_BinaryClassifier (CNN + 2-qubit circuit head).

Strategy: pure data-parallel over 8 cores (batch 65536 -> 8192/core).
Host-side: builds a conv1 im2col-transposed layout (patch-pixel on the
partition axis) in bf16 so conv1 becomes 12 dense [K=128 x M=128 x N]
matmuls per 512-image tile; all weights are baked into small DRAM tensors.
The quantum circuit + fc3 + log_softmax head is algebraically reduced to
6 cosines and 2 softplus per image.
"""
import os, sys
sys.path.insert(0, "/opt/trn_rl_repo")
import numpy as np
import ml_dtypes

from concourse import bass, tile, bacc
from concourse import mybir
from concourse.bass_utils import run_bass_kernel_spmd

dt = mybir.dt
AF = mybir.ActivationFunctionType
ALU = mybir.AluOpType

B = int(os.environ.get("BASS_KERNEL_B", "65536"))
NCORES = int(os.environ.get("BASS_KERNEL_CORES", "8"))
BC = B // NCORES          # 8192 images per core
NT = 512                  # images per tile
NTILES = BC // NT         # 16

# conv1 block grid: 12 blocks of 6y x 8x output pixels, patch 10x12 (=120 K rows)
BY, BX = 4, 3


def _bf16(a):
    return np.asarray(a, dtype=np.float32).astype(np.float16)


# ---------------------------------------------------------------- host packing

def build_im2col(x):
    """x: [B, 784] f32  ->  [12, 128, B] bf16 (k=120 bias row 1.0, 121..127 zero)."""
    n = x.shape[0]
    out = np.zeros((12, 128, n), dtype=np.float16)
    xb = _bf16(x)
    for by in range(BY):
        for bx in range(BX):
            b = by * 3 + bx
            y0, x0 = 6 * by, 8 * bx
            for iy in range(10):
                src = xb[:, (y0 + iy) * 28 + x0: (y0 + iy) * 28 + x0 + 12]
                out[b, iy * 12: iy * 12 + 12, :] = src.T
            out[b, 120, :] = np.float32(1.0)
    return out


def build_w1(w1, b1):
    """w1: [2,1,5,5], b1: [2] -> [128, 12*128] bf16 (lhsT per block)."""
    W = np.zeros((128, 12 * 128), dtype=np.float32)
    w1 = np.asarray(w1, dtype=np.float32).reshape(2, 5, 5)
    for by in range(BY):
        for bx in range(BX):
            b = by * 3 + bx
            for m in range(128):
                q, pay = m >> 5, m & 31
                if pay >= 24:
                    continue
                yp, xq = q >> 1, q & 1
                oc, r = pay // 12, pay % 12
                yy, xx = r // 4, r % 4
                dy, dx = 2 * yy + yp, 2 * xx + xq   # block-local out pixel
                for ky in range(5):
                    for kx in range(5):
                        iy, ix = dy + ky, dx + kx
                        W[iy * 12 + ix, b * 128 + m] = w1[oc, ky, kx]
                W[120, b * 128 + m] = b1[oc]
    return _bf16(W)


def build_w2(w2):
    """w2: [16,2,5,5] -> [128, 24*128] bf16; col ((j*3+g)*128 + m)."""
    W = np.zeros((128, 24 * 128), dtype=np.float32)
    w2 = np.asarray(w2, dtype=np.float32)
    for j in range(8):
        y2p, x2h = j >> 1, j & 1
        for g in range(3):
            for m in range(128):
                q2, pay2 = m >> 5, m & 31
                yp2, xp2 = q2 >> 1, q2 & 1
                oc2, x2l = pay2 >> 1, pay2 & 1
                y2 = 2 * y2p + yp2
                x2 = 2 * (2 * x2h + x2l) + xp2
                for p in range(128):
                    by, pay = p >> 5, p & 31
                    if pay >= 24:
                        continue
                    ch, r = pay // 12, pay % 12
                    yy, xx = r // 4, r % 4
                    prow, pcol = 3 * by + yy, 4 * g + xx
                    ky, kx = prow - y2, pcol - x2
                    if 0 <= ky < 5 and 0 <= kx < 5:
                        W[p, (j * 3 + g) * 128 + m] = w2[oc2, ch, ky, kx]
    return _bf16(W)


def build_wfc1(fc1_w):
    """fc1_w: [64, 256] -> [128, 2*64] bf16 (2 K-tiles)."""
    W = np.zeros((128, 2 * 64), dtype=np.float32)
    fc1_w = np.asarray(fc1_w, dtype=np.float32)
    for t in range(2):
        for p in range(128):
            quad, pay2 = p >> 5, p & 31
            j = 4 * t + quad
            y2p, x2h = j >> 1, j & 1
            oc2, x2l = pay2 >> 1, pay2 & 1
            flat = oc2 * 16 + y2p * 4 + (2 * x2h + x2l)
            W[p, t * 64: t * 64 + 64] = fc1_w[:, flat]
    return _bf16(W)


def head_constants(qnn_params, fc3_w, fc3_b, fc2_b):
    """Reduce the 2-qubit circuit tail + fc3 to z = c0 + sum Mk*cos(...)."""
    p = np.asarray(qnn_params, dtype=np.float64)

    def ry(t):
        c, s = np.cos(t), np.sin(t)
        return np.array([[c, -s], [s, c]])

    def kron_w0(U):  # on wire 0 (row index q0)
        return np.kron(U, np.eye(2))

    def kron_w1(U):
        return np.kron(np.eye(2), U)

    # state vector index = q0*2 + q1
    CN01 = np.zeros((4, 4)); CN01[0, 0] = CN01[1, 1] = 1; CN01[2, 3] = CN01[3, 2] = 1
    CN10 = np.zeros((4, 4)); CN10[0, 0] = 1; CN10[3, 1] = 1; CN10[1, 3] = 1; CN10[2, 2] = 1
    # CNOT control wire1 target wire0: (q0,q1)->(q0 xor q1, q1):
    CN10 = np.zeros((4, 4))
    for q0 in range(2):
        for q1 in range(2):
            CN10[((q0 ^ q1) * 2 + q1), q0 * 2 + q1] = 1
    U = np.eye(4)
    U = kron_w0(ry(p[0])) @ U
    U = kron_w1(ry(p[1])) @ U
    U = CN01 @ U
    U = kron_w0(ry(p[2])) @ U
    U = kron_w1(ry(p[3])) @ U
    U = CN10 @ U
    U = kron_w0(ry(p[4])) @ U
    U = kron_w1(ry(p[5])) @ U
    U = CN01 @ U
    U = kron_w0(ry(p[6])) @ U
    U = kron_w1(ry(p[7])) @ U
    S = np.diag([1.0, -1.0, -1.0, 1.0])
    M = 0.25 * (U.T @ S @ U)
    w3 = float(np.asarray(fc3_w).reshape(()))
    b3 = float(np.asarray(fc3_b).reshape(()))
    c0 = float(np.trace(M)) * w3 + b3
    # z = trace(M) + 2*[M01 cos(2x1+2ang) + M02 cos(2x0+2ang) + M03 cos(2x0+2x1)
    #                  + M12 cos(2x0-2x1) + M13 cos(2x0-2ang) + M23 cos(2x1-2ang)]
    k = {
        "A": 2 * M[0, 3] * w3,   # cos(2x0+2x1)
        "B": 2 * M[1, 2] * w3,   # cos(2x0-2x1)
        "C": 2 * M[0, 2] * w3,   # cos(2x0+2ang)
        "D": 2 * M[1, 3] * w3,   # cos(2x0-2ang)
        "E": 2 * M[0, 1] * w3,   # cos(2x1+2ang)
        "F": 2 * M[2, 3] * w3,   # cos(2x1-2ang)
    }
    return c0, k, float(fc2_b[0]), float(fc2_b[1])


# ---------------------------------------------------------------- bass program

def build_program(weights):
    nc = bacc.Bacc(None, target_bir_lowering=False, debug=False)
    a_c1 = nc.declare_dram_parameter("a_c1", [12, 128, BC], dt.float16, isOutput=False)
    w1_d = nc.declare_dram_parameter("w1", [128, 12 * 128], dt.float16, isOutput=False)
    w2_d = nc.declare_dram_parameter("w2", [128, 24 * 128], dt.float16, isOutput=False)
    wf1_d = nc.declare_dram_parameter("wf1", [128, 2 * 64], dt.float16, isOutput=False)
    wf2_d = nc.declare_dram_parameter("wf2", [64, 2], dt.float16, isOutput=False)
    cst_d = nc.declare_dram_parameter("cst", [128, 16], dt.float32, isOutput=False)
    y_d = nc.declare_dram_parameter("y", [2, BC], dt.float32, isOutput=True)
    DBG = bool(int(os.environ.get("BASS_KERNEL_DEBUG", "0")))
    if DBG:
        dC1 = nc.declare_dram_parameter("dC1", [128, 12 * NT], dt.float16, isOutput=True)
        dE1 = nc.declare_dram_parameter("dE1", [128, 3 * NT], dt.float16, isOutput=True)
        dC2 = nc.declare_dram_parameter("dC2", [128, 8 * NT], dt.float16, isOutput=True)
        dE2 = nc.declare_dram_parameter("dE2", [128, 2 * NT], dt.float16, isOutput=True)
        dF1 = nc.declare_dram_parameter("dF1", [64, NT], dt.float16, isOutput=True)
        dH = nc.declare_dram_parameter("dH", [128, 8], dt.float32, isOutput=True)
        dY = nc.declare_dram_parameter("dY", [128, 4], dt.float32, isOutput=True)

    c0, K, b20, b21 = weights["head"]
    pi = float(np.pi)
    # consts columns: 0:bias2(conv2 per-partition), 1:fc1 bias, 2..7: sin biases
    # sin arg biases (scale=2 on raw sums):  cos(u) = sin(u + pi/2)
    sin_bias = {
        "A": pi / 2 + 2 * (b20 + b21),
        "B": pi / 2 + 2 * (b20 - b21),
        "C": pi / 2 + 2 * b20,   # + 2*ang handled in operand
        "D": pi / 2 + 2 * b20,
        "E": pi / 2 + 2 * b21,
        "F": pi / 2 + 2 * b21,
    }

    with tile.TileContext(nc) as tc:
        with tc.tile_pool(name="cw", bufs=1) as cw, \
             tc.tile_pool(name="sx", bufs=2) as sx, \
             tc.tile_pool(name="sa", bufs=2) as sa, \
             tc.tile_pool(name="mid", bufs=1) as mid, \
             tc.tile_pool(name="sb2", bufs=2) as sb2, \
             tc.tile_pool(name="ph", bufs=2) as ph, \
             tc.tile_pool(name="p1", bufs=3, space="PSUM") as p1, \
             tc.tile_pool(name="p2", bufs=2, space="PSUM") as p2, \
             tc.tile_pool(name="pf", bufs=1, space="PSUM") as pf, \
             tc.tile_pool(name="pg", bufs=1, space="PSUM") as pg:

            W1 = cw.tile([128, 12 * 128], dt.float16)
            nc.sync.dma_start(out=W1[:], in_=w1_d[:])
            W2 = cw.tile([128, 24 * 128], dt.float16)
            nc.sync.dma_start(out=W2[:], in_=w2_d[:])
            WF1 = cw.tile([128, 2 * 64], dt.float16)
            nc.sync.dma_start(out=WF1[:], in_=wf1_d[:])
            WF2 = cw.tile([64, 2], dt.float16)
            nc.sync.dma_start(out=WF2[:], in_=wf2_d[:])
            CST = cw.tile([128, 16], dt.float32)
            nc.sync.dma_start(out=CST[:], in_=cst_d[:])
            Yall = cw.tile([128, NTILES * 4], dt.float32)

            for it in range(NTILES):
                n0 = it * NT
                # ---- load conv1 im2col tile [128, 12*NT]
                xc = sx.tile([128, 12 * NT], dt.float16)
                nc.sync.dma_start(
                    out=xc[:].rearrange("p (b n) -> p b n", b=12),
                    in_=a_c1[:, :, n0:n0 + NT].transpose([1, 0, 2]))

                # ---- conv1: 12 matmuls -> drain(relu,bf16) -> C1
                C1 = sa.tile([128, 12 * NT], dt.float16, tag="C1")
                for b in range(12):
                    ps = p1.tile([128, NT], dt.float32, tag="psc1")
                    nc.tensor.matmul(out=ps[:], lhsT=W1[:, b * 128:(b + 1) * 128],
                                     rhs=xc[:, b * NT:(b + 1) * NT],
                                     start=True, stop=True)
                    nc.scalar.activation(out=C1[:, b * NT:(b + 1) * NT], in_=ps[:],
                                         func=AF.Relu)

                # ---- pool1: shift -> max -> shift -> max (quad-placed)
                S1 = mid.tile([64, 12 * NT], dt.float16, tag="S1")
                nc.any.tensor_copy(out=S1[:], in_=C1[64:128, :])
                M1 = mid.tile([64, 12 * NT], dt.float16, tag="M1")
                nc.any.tensor_tensor(out=M1[:], in0=C1[0:64, :], in1=S1[:], op=ALU.max)
                S2 = mid.tile([32, 12 * NT], dt.float16, tag="S2")
                nc.any.tensor_copy(out=S2[:], in_=M1[32:64, :])
                E1 = sa.tile([128, 3 * NT], dt.float16, tag="E1")
                for by in range(BY):
                    nc.any.tensor_tensor(
                        out=E1[32 * by:32 * by + 32, :],
                        in0=M1[0:32, by * 3 * NT:(by + 1) * 3 * NT],
                        in1=S2[:, by * 3 * NT:(by + 1) * 3 * NT], op=ALU.max)

                # ---- conv2: 8 M-blocks x 3 accumulated matmuls
                C2 = sa.tile([128, 8 * NT], dt.float16, tag="C2")
                for j in range(8):
                    ps2 = p2.tile([128, NT], dt.float32, tag="psc2")
                    for g in range(3):
                        nc.tensor.matmul(out=ps2[:],
                                         lhsT=W2[:, (j * 3 + g) * 128:(j * 3 + g + 1) * 128],
                                         rhs=E1[:, g * NT:(g + 1) * NT],
                                         start=(g == 0), stop=(g == 2))
                    nc.scalar.activation(out=C2[:, j * NT:(j + 1) * NT], in_=ps2[:],
                                         func=AF.Relu, bias=CST[:, 0:1])

                # ---- pool2
                S1b = mid.tile([64, 8 * NT], dt.float16, tag="S1b")
                nc.any.tensor_copy(out=S1b[:], in_=C2[64:128, :])
                M2 = mid.tile([64, 8 * NT], dt.float16, tag="M2")
                nc.any.tensor_tensor(out=M2[:], in0=C2[0:64, :], in1=S1b[:], op=ALU.max)
                S2b = mid.tile([32, 8 * NT], dt.float16, tag="S2b")
                nc.any.tensor_copy(out=S2b[:], in_=M2[32:64, :])
                E2_0 = sb2.tile([128, NT], dt.float16, tag="E2_0")
                E2_1 = sb2.tile([128, NT], dt.float16, tag="E2_1")
                for j in range(8):
                    dst = E2_0 if j < 4 else E2_1
                    quad = j % 4
                    nc.any.tensor_tensor(
                        out=dst[32 * quad:32 * quad + 32, :],
                        in0=M2[0:32, j * NT:(j + 1) * NT],
                        in1=S2b[:, j * NT:(j + 1) * NT], op=ALU.max)

                # ---- fc1 (K=256 over 2 tiles) -> relu -> F1 bf16
                psf = pf.tile([64, NT], dt.float32, tag="psf")
                nc.tensor.matmul(out=psf[:], lhsT=WF1[:, 0:64], rhs=E2_0[:],
                                 start=True, stop=False)
                nc.tensor.matmul(out=psf[:], lhsT=WF1[:, 64:128], rhs=E2_1[:],
                                 start=False, stop=True)
                F1 = sb2.tile([64, NT], dt.float16, tag="F1")
                nc.scalar.activation(out=F1[:], in_=psf[:], func=AF.Relu,
                                     bias=CST[0:64, 1:2])

                # ---- fc2 img-major: 4 matmuls N=2 -> H [128, (s,2)] f32
                psg = pg.tile([128, 8], dt.float32, tag="psg")
                for s in range(4):
                    nc.tensor.matmul(out=psg[:, 2 * s:2 * s + 2],
                                     lhsT=F1[:, 128 * s:128 * (s + 1)],
                                     rhs=WF2[:], start=True, stop=True)
                H = ph.tile([128, 8], dt.float32, tag="H")
                nc.vector.tensor_copy(out=H[:], in_=psg[:])

                # ---- head: x0 = H[:, s, 0], x1 = H[:, s, 1] (raw, biases folded)
                Hv = H[:].rearrange("p (s c) -> p s c", c=2)
                x0 = Hv[:, :, 0]
                x1 = Hv[:, :, 1]
                t0 = ph.tile([128, 4], dt.float32, tag="t0")
                nc.any.tensor_scalar(out=t0[:], in0=x0, scalar1=-1.0, scalar2=pi - b20,
                                     op0=ALU.mult, op1=ALU.add)
                t1 = ph.tile([128, 4], dt.float32, tag="t1")
                nc.any.tensor_scalar(out=t1[:], in0=x1, scalar1=-1.0, scalar2=pi - b21,
                                     op0=ALU.mult, op1=ALU.add)
                ang = ph.tile([128, 4], dt.float32, tag="ang")
                nc.any.tensor_tensor(out=ang[:], in0=t0[:], in1=t1[:], op=ALU.mult)

                # args packed into AR [128, (k:6, 4)]; AR_k = raw_k + halfbias_k
                # so that cos-term_k = sin(2*AR_k) after range reduction.
                qpi = pi / 4
                hb = {"A": b20 + b21 + qpi, "B": b20 - b21 + qpi,
                      "C": b20 + qpi, "D": b20 + qpi,
                      "E": b21 + qpi, "F": b21 + qpi}
                AR = ph.tile([128, 24], dt.float32, tag="AR")
                plan = (("A", x0, x1, ALU.add), ("B", x0, x1, ALU.subtract),
                        ("C", x0, ang[:], ALU.add), ("D", x0, ang[:], ALU.subtract),
                        ("E", x1, ang[:], ALU.add), ("F", x1, ang[:], ALU.subtract))
                for i, (nm, a0, a1, op) in enumerate(plan):
                    nc.vector.scalar_tensor_tensor(
                        out=AR[:, 4 * i:4 * i + 4], in0=a0, scalar=hb[nm],
                        in1=a1, op0=ALU.add, op1=op)
                # range reduce: h = AR - pi*round(AR/pi); sin(2h) = sin(2AR mod 2pi)
                tq = ph.tile([128, 24], dt.float32, tag="tq")
                nc.any.tensor_scalar(out=tq[:], in0=AR[:], scalar1=float(1 / pi),
                                     scalar2=None, op0=ALU.mult)
                ti_ = ph.tile([128, 24], dt.int32, tag="ti")
                nc.vector.tensor_copy(out=ti_[:], in_=tq[:])
                tf_ = ph.tile([128, 24], dt.float32, tag="tf")
                nc.vector.tensor_copy(out=tf_[:], in_=ti_[:])
                hh = ph.tile([128, 24], dt.float32, tag="hh")
                nc.vector.scalar_tensor_tensor(out=hh[:], in0=tf_[:], scalar=-pi,
                                               in1=AR[:], op0=ALU.mult, op1=ALU.add)
                SN = ph.tile([128, 24], dt.float32, tag="SN")
                nc.scalar.activation(out=SN[:], in_=hh[:], func=AF.Sin, scale=2.0)
                cosv = {nm: SN[:, 4 * i:4 * i + 4]
                        for i, nm in enumerate("ABCDEF")}

                acc = ph.tile([128, 4], dt.float32, tag="acc0")
                nc.any.tensor_scalar(out=acc[:], in0=cosv["A"], scalar1=K["A"],
                                     scalar2=c0, op0=ALU.mult, op1=ALU.add)
                for nm in "BCDE":
                    acc2 = ph.tile([128, 4], dt.float32, tag="accn")
                    nc.vector.scalar_tensor_tensor(out=acc2[:], in0=cosv[nm][:],
                                                   scalar=K[nm], in1=acc[:],
                                                   op0=ALU.mult, op1=ALU.add)
                    acc = acc2
                nc.vector.scalar_tensor_tensor(out=Yall[:, it * 4:(it + 1) * 4],
                                               in0=cosv["F"], scalar=K["F"],
                                               in1=acc[:], op0=ALU.mult, op1=ALU.add)

                if DBG and it == 0:
                    for dd, tt_ in ((dC1, C1), (dE1, E1), (dC2, C2), (dF1, F1)):
                        nc.sync.dma_start(out=dd[:], in_=tt_[:])
                    nc.sync.dma_start(out=dH[:], in_=H[:])
                    nc.sync.dma_start(out=dE2[:, 0:NT], in_=E2_0[:])
                    nc.sync.dma_start(out=dE2[:, NT:2 * NT], in_=E2_1[:])
                    nc.sync.dma_start(out=dY[:], in_=Yall[:, 0:4])

            # ---- final head: out0=-ln(1+e^{1-2y}), out1=-ln(1+e^{2y-1})
            V = cw.tile([128, NTILES * 4], dt.float32)
            nc.scalar.activation(out=V[:], in_=Yall[:], func=AF.Exp,
                                 bias=CST[:, 8:9], scale=-2.0)      # e^{1-2y}
            Wr = cw.tile([128, NTILES * 4], dt.float32)
            nc.vector.reciprocal(out=Wr[:], in_=V[:])               # e^{2y-1}
            L0 = cw.tile([128, NTILES * 4], dt.float32)
            nc.scalar.activation(out=L0[:], in_=V[:], func=AF.Ln,
                                 bias=CST[:, 8:9], scale=1.0)       # ln(1+v)
            L1 = cw.tile([128, NTILES * 4], dt.float32)
            nc.scalar.activation(out=L1[:], in_=Wr[:], func=AF.Ln,
                                 bias=CST[:, 8:9], scale=1.0)
            O = cw.tile([128, NTILES * 8], dt.float32)
            Ov = O[:].rearrange("p (c n) -> p c n", c=2)
            nc.any.tensor_scalar(out=Ov[:, 0, :], in0=L0[:], scalar1=-1.0,
                                 scalar2=None, op0=ALU.mult)
            nc.any.tensor_scalar(out=Ov[:, 1, :], in0=L1[:], scalar1=-1.0,
                                 scalar2=None, op0=ALU.mult)
            # y layout [2, BC]: dst[c, it*512+s*128+p] <- O[p, (c, it, s)]
            for c in range(2):
                nc.sync.dma_start(
                    out=y_d[c, :].rearrange("(t s p) -> p t s", p=128, s=4),
                    in_=Ov[:, c, :].rearrange("p (t s) -> p t s", s=4))

    nc.compile()
    return nc


_PROG_CACHE = {}


def kernel(x, conv1_w, conv1_b, conv2_w, conv2_b, fc1_w, fc1_b,
           fc2_w, fc2_b, fc3_w, fc3_b, qnn_params):
    x = np.asarray(x, dtype=np.float32).reshape(B, 784)
    a = build_im2col(x.reshape(B, 784))          # [12,128,B] bf16
    W1 = build_w1(conv1_w, np.asarray(conv1_b, np.float32))
    W2 = build_w2(conv2_w)
    WF1 = build_wfc1(fc1_w)
    WF2 = _bf16(np.asarray(fc2_w, np.float32).T)  # [64, 2]
    c0, K, b20, b21 = head_constants(qnn_params, fc3_w, fc3_b,
                                     np.asarray(fc2_b, np.float32))
    pi = float(np.pi)
    cst = np.zeros((128, 16), dtype=np.float32)
    b2 = np.asarray(conv2_b, np.float32)
    for p in range(128):
        cst[p, 0] = b2[(p & 31) >> 1]            # conv2 bias per partition
    cst[0:64, 1] = np.asarray(fc1_b, np.float32)
    sin_bias = [pi / 2 + 2 * (b20 + b21), pi / 2 + 2 * (b20 - b21),
                pi / 2 + 2 * b20, pi / 2 + 2 * b20,
                pi / 2 + 2 * b21, pi / 2 + 2 * b21]
    for i in range(6):
        cst[:, 2 + i] = sin_bias[i]
    cst[:, 8] = 1.0     # softplus bias for out0: softplus(-2y + 1)
    cst[:, 9] = -1.0    # softplus bias for out1: softplus(2y - 1)

    weights = {"head": (c0, K, b20, b21)}
    nc = build_program(weights)

    in_maps = []
    for c in range(NCORES):
        sl = slice(c * BC, (c + 1) * BC)
        in_maps.append({
            "a_c1": np.ascontiguousarray(a[:, :, sl]),
            "w1": W1, "w2": W2, "wf1": WF1, "wf2": WF2, "cst": cst,
        })
    res = run_bass_kernel_spmd(nc, in_maps, list(range(NCORES)),
                               trace=bool(int(os.environ.get("BASS_TRACE_KERNEL", "0"))))
    if res.exec_time_ns is not None:
        print(f"HW exec time: {res.exec_time_ns} ns")
    global LAST_RESULTS
    LAST_RESULTS = res.results
    out = np.empty((B, 2), dtype=np.float32)
    for c in range(NCORES):
        out[c * BC:(c + 1) * BC] = res.results[c]["y"].T
    return out

